# revision 11
# baseline (speedup 1.0000x reference)
"""Trainium2 Bass kernel for nn_Block_75840532513234 (dense transformer block).

Sharding: 8 cores; core c handles batch b = c//2 and head-half hh = c%2
(8 of 16 heads). The two cores of a pair all-reduce the c_proj partial sums
(row-sharded c_proj) after each of the two sub-blocks, so both hold the full
residual stream; the pair's even core's output is used.

Layout strategy (per core):
  - x, x' kept natural (t-part, c-free); LN stats per-partition.
  - LN mean/scale folded into projection evictions as a rank-1 correction
    ((x@W - mu*colsum(W)) * rstd), so x is transposed raw on the PE.
  - LeakyAvg = banded matmul with two 128x128 per-head decay matrices
    (exp decay below ~1e-28 beyond 128 steps for beta >= 0.5).
  - Attention computed fully transposed: scoresT = KT.T-block @ KT,
    attT = exp(scoresT) (softmax needs no max subtraction: |scores| <= ~10),
    YT = [V|1].T @ attT gives Y and the denominators in one accumulation.
  - ln1/ln2 weights folded into W_k/W_v host-side; kscale via svec;
    out_scale/PM_COUNT folded into Pv host-side; vscale into v-normalize.
"""
import sys
import math

sys.path.insert(0, '/opt/trn_rl_repo')

import numpy as np

# ---------------------------------------------------------------------------
# Patches for this container's walrus build: it allows only ONE sync-wait per
# instruction, while Tile attaches several (final drain; ldweights). Split the
# extras onto standalone single-wait EventSemaphore instructions.
# ---------------------------------------------------------------------------
import concourse.tile as tile
import concourse.bass as bass
from concourse import mybir
from concourse.vector_clock import ScopedClock

_ev_ctr = [0]


def _split_multi_waits(nc):
    for f in nc.m.functions:
        for bb in f.blocks:
            il = bb.instructions
            i = 0
            while i < len(il):
                inst = il[i]
                si = inst.sync_info
                if si is not None and si.on_wait and len(si.on_wait) > 1:
                    waits = list(si.on_wait)
                    si.on_wait.clear()
                    si.on_wait.append(waits[-1])
                    for w in waits[:-1]:
                        _ev_ctr[0] += 1
                        ev = mybir.InstEventSemaphore(
                            name=f"EVSPLIT-{_ev_ctr[0]}", ins=[], outs=[])
                        ev.engine = inst.engine
                        ev.sync_info = mybir.SyncInfo(on_wait=[], on_update=[])
                        ev.sync_info.on_wait.append(w)
                        il.insert(i, ev)
                        i += 1
                i += 1


def _patched_drain_and_barrier(self, tick_clock, wait_clock):
    nc = self.nc
    drain_inst = nc.sync.drain()
    wait_clock.add_sem_waits(
        drain_inst.ins, ScopedClock({None: tick_clock.global_clock}))
    nc.all_engine_barrier()
    popped = nc._tile_sem_poison_stack.pop()
    assert popped is self._sem_poison
    nc.clear_and_free_semaphores(list(self.sems.allocated().values()))
    nc.all_engine_barrier()


tile.TileContext._drain_and_barrier = _patched_drain_and_barrier

_orig_tile_exit = tile.TileContext.__exit__


def _patched_tile_exit(self, *a, **k):
    r = _orig_tile_exit(self, *a, **k)
    _split_multi_waits(self.nc)
    return r


tile.TileContext.__exit__ = _patched_tile_exit

# NTFF profile hook (trimmed image lacks antenv.axon_hooks).
import types as _types

if "antenv.axon_hooks" not in sys.modules:
    _m = _types.ModuleType("antenv.axon_hooks")
    _hook_store = [None]

    def _set_hook(h):
        _hook_store[0] = h

    def _get_hook():
        if _hook_store[0] is None:
            try:
                if '/root/.axon_site' not in sys.path:
                    sys.path.insert(0, '/root/.axon_site')
                from trn_agent_boot.trn_boot import _ntff_profile_via_ctypes
                _hook_store[0] = _ntff_profile_via_ctypes(
                    '/opt/axon/libaxon_pjrt.so')
            except Exception:
                return None
        return _hook_store[0]

    _m.set_axon_ntff_profile_hook = _set_hook
    _m.get_axon_ntff_profile_hook = _get_hook
    sys.modules["antenv.axon_hooks"] = _m
    import antenv as _antenv
    _antenv.axon_hooks = _m

from concourse.bass_utils import run_bass_kernel_spmd  # noqa: E402

# ---------------------------------------------------------------------------
# Problem constants (hardcoded per the grading contract)
# ---------------------------------------------------------------------------
B, T, C, NH = 4, 1024, 1024, 16
HS = C // NH              # 64
NHL = 8                   # heads per core
CL = NHL * HS             # 512 local channels
EXP_SCALING = 10.0
KSCALE_MAX = float(np.log(2.0 ** 16 - 1))
N_CORES = 8
GROUPS = [[0, 1], [2, 3], [4, 5], [6, 7]]

f32 = mybir.dt.float32
f32r = mybir.dt.float32r
AF = mybir.ActivationFunctionType
ALU = mybir.AluOpType
AXL = mybir.AxisListType

NTB = T // 128            # 8 t-blocks
NCB = C // 128            # 8 c-blocks


def _build_program():
    nc = bass.Bass(num_devices=N_CORES)

    # ---- I/O ----
    x_in = nc.dram_tensor("x", [T, C], f32, kind="ExternalInput")
    wka_in = nc.dram_tensor("wka", [C, CL], f32, kind="ExternalInput")
    wva_in = nc.dram_tensor("wva", [C, CL], f32, kind="ExternalInput")
    wkm_in = nc.dram_tensor("wkm", [C, CL], f32, kind="ExternalInput")
    cpa_in = nc.dram_tensor("cpa", [CL, C], f32, kind="ExternalInput")
    cpm_in = nc.dram_tensor("cpm", [CL, C], f32, kind="ExternalInput")
    wkasb_in = nc.dram_tensor("wkasb", [128, CL], f32, kind="ExternalInput")
    wvasb_in = nc.dram_tensor("wvasb", [128, CL], f32, kind="ExternalInput")
    wkmsb_in = nc.dram_tensor("wkmsb", [128, CL], f32, kind="ExternalInput")
    dmata_in = nc.dram_tensor("dmata", [128, NHL * 256], f32, kind="ExternalInput")
    dmatm_in = nc.dram_tensor("dmatm", [128, NHL * 256], f32, kind="ExternalInput")
    sveca_in = nc.dram_tensor("sveca", [128, NHL], f32, kind="ExternalInput")
    svecm_in = nc.dram_tensor("svecm", [128, NHL], f32, kind="ExternalInput")
    coef1_in = nc.dram_tensor("coef1", [128, NHL], f32, kind="ExternalInput")
    vs_in = nc.dram_tensor("vs", [128, NHL], f32, kind="ExternalInput")
    pkt_in = nc.dram_tensor("pkt", [128, 4096], f32, kind="ExternalInput")
    pvo_in = nc.dram_tensor("pvo", [128, NHL * 8 * 66], f32, kind="ExternalInput")
    ident_in = nc.dram_tensor("ident", [128, 128], f32, kind="ExternalInput")
    maskt_in = nc.dram_tensor("maskt", [128, 4 * 512], f32, kind="ExternalInput")
    ones1_in = nc.dram_tensor("ones1", [1, 64], f32, kind="ExternalInput")
    vpad_in = nc.dram_tensor("vpad", [128, 16], f32, kind="ExternalInput")
    zcol_in = nc.dram_tensor("zcol", [128, 1], f32, kind="ExternalInput")
    epsv_in = nc.dram_tensor("epsv", [128, 1], f32, kind="ExternalInput")
    zrow_in = nc.dram_tensor("zrow", [1, 512], f32, kind="ExternalInput")

    out_t = nc.dram_tensor("out", [T, C], f32, kind="ExternalOutput")

    cc1_in = nc.dram_tensor("cc1_in", [T, C], f32)
    cc1_out = nc.dram_tensor("cc1_out", [T, C], f32)
    cc2_in = nc.dram_tensor("cc2_in", [T, C], f32)
    cc2_out = nc.dram_tensor("cc2_out", [T, C], f32)

    with tile.TileContext(nc) as tc:
        # ---------------- persistent pools ----------------
        with tc.tile_pool(name="persist", bufs=1) as pp, \
             tc.tile_pool(name="work", bufs=1) as wp:
            # constants
            ident = pp.tile([128, 128], f32r, tag="ident")
            nc.sync.dma_start(ident[:], ident_in[:].bitcast(f32r))
            sveca = pp.tile([128, NHL], f32, tag="sveca")
            nc.sync.dma_start(sveca[:], sveca_in[:])
            svecm = pp.tile([128, NHL], f32, tag="svecm")
            nc.sync.dma_start(svecm[:], svecm_in[:])
            coef1 = pp.tile([128, NHL], f32, tag="coef1")
            nc.sync.dma_start(coef1[:], coef1_in[:])
            vsv = pp.tile([128, NHL], f32, tag="vsv")
            nc.sync.dma_start(vsv[:], vs_in[:])
            ones1 = pp.tile([1, 64], f32r, tag="ones1")
            nc.sync.dma_start(ones1[:], ones1_in[:].bitcast(f32r))
            vpad = pp.tile([128, 16], f32r, tag="vpad")
            nc.sync.dma_start(vpad[:], vpad_in[:].bitcast(f32r))
            zcol = pp.tile([128, 1], f32r, tag="zcol")
            nc.sync.dma_start(zcol[:], zcol_in[:].bitcast(f32r))
            epsv = pp.tile([128, 1], f32, tag="epsv")
            nc.sync.dma_start(epsv[:], epsv_in[:])
            maskt = pp.tile([128, 2048], f32, tag="maskt")
            nc.sync.dma_start(maskt[:], maskt_in[:])
            pvo = pp.tile([128, NHL * 8 * 66], f32r, tag="pvo")
            nc.sync.dma_start(pvo[:], pvo_in[:].bitcast(f32r))
            wsb = {}
            for nm, src in (("wka", wkasb_in), ("wva", wvasb_in),
                            ("wkm", wkmsb_in)):
                wsb[nm] = pp.tile([128, CL], f32, name=f"wsb_{nm}", tag=f"wsb_{nm}")
                nc.sync.dma_start(wsb[nm][:], src[:])

            # big persistent buffers (tags reused across the two branches)
            # X is f32r so PE can transpose it directly (rounds x once, ~1e-4)
            X = wp.tile([128, NTB * 1024], f32r, tag="X")        # x then out
            nc.sync.dma_start(
                X[:].rearrange("p (b c) -> p b c", b=NTB),
                x_in[:].bitcast(f32r).rearrange("(b p) c -> p b c", p=128))

            def branch(branch_id, wk_in, wsum_bc, dmat_in, svec, cc_in_t,
                       cc_out_t, wv_in=None, wvsum_bc=None, cp_in=None):
                """Emit one sub-block (context: with v; persistent: without).

                Returns nothing; accumulates residual into X in place.
                branch uses tiles tagged with shared tags so the second call
                reuses the first call's SBUF.
                """
                is_ctx = wv_in is not None

                # ---- LN stats ----
                negmu = []
                rstd = []
                with tc.tile_pool(name=f"st{branch_id}", bufs=2 * NTB) as sp, \
                     tc.tile_pool(name=f"sttmp{branch_id}", bufs=2) as stp:
                    for tb in range(NTB):
                        xs = X[:, tb * 1024:(tb + 1) * 1024].bitcast(f32)
                        s = stp.tile([128, 1], f32, tag="s")
                        nc.vector.reduce_sum(s[:], xs, axis=AXL.X)
                        sq = stp.tile([128, 1024], f32, tag="sq")
                        nc.vector.tensor_tensor(sq[:], xs, xs, ALU.mult)
                        ssq = stp.tile([128, 1], f32, tag="ssq")
                        nc.vector.reduce_sum(ssq[:], sq[:], axis=AXL.X)
                        nm = sp.tile([128, 1], f32, tag="negmu")
                        nc.vector.tensor_scalar_mul(nm[:], s[:], -1.0 / C)
                        mp = stp.tile([128, 1], f32, tag="mp")
                        nc.vector.tensor_scalar_mul(mp[:], s[:], 1.0 / C)
                        sn = stp.tile([128, 1], f32, tag="sn")
                        nc.vector.tensor_scalar_mul(sn[:], ssq[:], 1.0 / C)
                        var = stp.tile([128, 1], f32, tag="var")
                        nc.vector.scalar_tensor_tensor(
                            var[:], mp[:], nm[:], sn[:], ALU.mult, ALU.add)
                        lv = stp.tile([128, 1], f32, tag="lv")
                        nc.scalar.activation(lv[:], var[:], AF.Ln, bias=epsv[:])
                        rs = sp.tile([128, 1], f32, tag="rstd")
                        nc.scalar.activation(rs[:], lv[:], AF.Exp, scale=-0.5)
                        negmu.append(nm)
                        rstd.append(rs)

                    # ---- transpose + projections (grouped for PSUM) ----
                    kraw = wp.tile([128, NTB * CL], f32r, tag="kraw")
                    vraw = None
                    if is_ctx:
                        vraw = wp.tile([128, NTB * CL], f32, tag="vraw")
                    tgroups = [(0, 3), (3, 6), (6, 8)]
                    with tc.tile_pool(name=f"pj{branch_id}", bufs=1,
                                      space="PSUM") as pjp, \
                         tc.tile_pool(name=f"ptr{branch_id}", bufs=2,
                                      space="PSUM") as ptp, \
                         tc.tile_pool(name=f"w{branch_id}", bufs=4) as wpool, \
                         tc.tile_pool(name=f"xtc{branch_id}", bufs=2) as xtp, \
                         tc.tile_pool(name=f"ev{branch_id}", bufs=3) as evp:
                        for g0, g1 in tgroups:
                            gn = g1 - g0
                            psk = [pjp.tile([128, CL], f32,
                                            name=f"psk{g0}_{i}",
                                            tag=f"psk{i}")
                                   for i in range(gn)]
                            psv = [pjp.tile([128, CL], f32,
                                            name=f"psv{g0}_{i}",
                                            tag=f"psv{i}")
                                   for i in range(gn)] if is_ctx else None
                            for cb in range(NCB):
                                xtc = xtp.tile([128, gn * 128], f32r,
                                               tag="xtc")
                                for i, tb in enumerate(range(g0, g1)):
                                    ptr = ptp.tile([128, 128], f32, tag="ptr")
                                    nc.tensor.transpose(
                                        ptr[:].bitcast(f32r),
                                        X[:, tb * 1024 + cb * 128:
                                          tb * 1024 + cb * 128 + 128],
                                        ident[:])
                                    nc.scalar.copy(
                                        xtc[:, i * 128:(i + 1) * 128],
                                        ptr[:])
                                wk_c = wpool.tile([128, CL], f32r, tag="wk")
                                nc.sync.dma_start(
                                    wk_c[:],
                                    wk_in[cb * 128:(cb + 1) * 128, :]
                                    .bitcast(f32r))
                                wv_c = None
                                if is_ctx:
                                    wv_c = wpool.tile([128, CL], f32r,
                                                      tag="wv")
                                    nc.sync.dma_start(
                                        wv_c[:],
                                        wv_in[cb * 128:(cb + 1) * 128, :]
                                        .bitcast(f32r))
                                for i in range(gn):
                                    nc.tensor.matmul(
                                        psk[i][:],
                                        xtc[:, i * 128:(i + 1) * 128],
                                        wk_c[:], start=(cb == 0),
                                        stop=(cb == NCB - 1))
                                    if is_ctx:
                                        nc.tensor.matmul(
                                            psv[i][:],
                                            xtc[:, i * 128:(i + 1) * 128],
                                            wv_c[:], start=(cb == 0),
                                            stop=(cb == NCB - 1))
                            # evict with LN fold: (psum + negmu*wsum)*rstd
                            for i, tb in enumerate(range(g0, g1)):
                                tmp = evp.tile([128, CL], f32, tag="evt")
                                nc.vector.scalar_tensor_tensor(
                                    tmp[:], wsum_bc[:], negmu[tb][:],
                                    psk[i][:], ALU.mult, ALU.add)
                                nc.vector.tensor_scalar_mul(
                                    kraw[:, tb * CL:(tb + 1) * CL],
                                    tmp[:], rstd[tb][:])
                                if is_ctx:
                                    tmp2 = evp.tile([128, CL], f32,
                                                    tag="evt2")
                                    nc.vector.scalar_tensor_tensor(
                                        tmp2[:], wvsum_bc[:], negmu[tb][:],
                                        psv[i][:], ALU.mult, ALU.add)
                                    nc.vector.tensor_scalar_mul(
                                        vraw[:, tb * CL:(tb + 1) * CL],
                                        tmp2[:], rstd[tb][:])

                # ---- v path (context only) ----
                vnorm = None
                if is_ctx:
                    vnorm = wp.tile([128, NTB * 528], f32r, tag="vnorm")
                    with tc.tile_pool(name="vtmp", bufs=1) as vtp, \
                         tc.tile_pool(name="vst", bufs=1) as vsp:
                        FS = NTB * CL
                        vsh = vtp.tile([128, FS], f32, tag="vsh")
                        # v_shift via DMA: partition +1, wrap to next t-block
                        nc.sync.dma_start(vsh[0:127, :], vraw[1:128, :])
                        nc.sync.dma_start(vsh[127:128, 0:FS - CL],
                                          vraw[0:1, CL:FS])
                        nc.sync.dma_start(vsh[127:128, FS - CL:FS],
                                          zrow_in[:])
                        # diff/vmix computed in place in vsh
                        nc.vector.tensor_tensor(vsh[:], vsh[:], vraw[:],
                                                ALU.subtract)
                        c_b = coef1[:].unsqueeze(1).unsqueeze(3).broadcast_to(
                            (128, NTB, NHL, HS))
                        nc.vector.tensor_tensor(
                            vsh[:].rearrange("p (b h d) -> p b h d", b=NTB,
                                             h=NHL),
                            vsh[:].rearrange("p (b h d) -> p b h d", b=NTB,
                                             h=NHL),
                            c_b, ALU.mult)
                        nc.vector.tensor_tensor(vsh[:], vsh[:], vraw[:],
                                                ALU.add)
                        vmix = vsh
                        sq = vtp.tile([128, FS], f32, tag="vsq")
                        nc.vector.tensor_tensor(sq[:], vmix[:], vmix[:],
                                                ALU.mult)
                        ssq = vsp.tile([128, NTB * NHL], f32, tag="vssq")
                        nc.vector.reduce_sum(
                            ssq[:], sq[:].rearrange("p (g d) -> p g d", d=HS),
                            axis=AXL.X)
                        lnv = vsp.tile([128, NTB * NHL], f32, tag="vlnv")
                        nc.scalar.activation(lnv[:], ssq[:], AF.Ln)
                        rn = vsp.tile([128, NTB * NHL], f32, tag="vrn")
                        nc.scalar.activation(rn[:], lnv[:], AF.Exp,
                                             scale=-0.5)
                        rns = vsp.tile([128, NTB * NHL], f32, tag="vrns")
                        vs_b = vsv[:].unsqueeze(1).broadcast_to(
                            (128, NTB, NHL))
                        nc.vector.tensor_tensor(
                            rns[:].rearrange("p (b h) -> p b h", b=NTB),
                            rn[:].rearrange("p (b h) -> p b h", b=NTB),
                            vs_b, ALU.mult)
                        rns_b = rns[:].rearrange(
                            "p (b h) -> p b h", b=NTB).unsqueeze(
                            3).broadcast_to((128, NTB, NHL, HS))
                        nc.vector.tensor_tensor(
                            vnorm[:].rearrange("p (b h c) -> p b h c", b=NTB,
                                               h=NHL)[:, :, :, 0:64],
                            vmix[:].rearrange("p (b h d) -> p b h d", b=NTB,
                                              h=NHL),
                            rns_b, ALU.mult)
                        # ones/zeros pad columns for all t-blocks at once
                        vp_b = vpad[:].rearrange("p (h t) -> p h t", h=NHL)
                        for tb in range(NTB):
                            nc.sync.dma_start(
                                vnorm[:, tb * 528:(tb + 1) * 528].rearrange(
                                    "p (h c) -> p h c", h=NHL)[:, :, 64:66],
                                vp_b)

                # ---- LeakyAvg + normalize + transpose -> ktall ----
                dmat = wp.tile([128, NHL * 256], f32r, tag="dmat")
                nc.sync.dma_start(dmat[:], dmat_in[:].bitcast(f32r))
                ktall = wp.tile([128, 4096], f32r, tag="ktall")
                with tc.tile_pool(name=f"lv{branch_id}", bufs=2,
                                  space="PSUM") as lvp, \
                     tc.tile_pool(name=f"ltr{branch_id}", bufs=2,
                                  space="PSUM") as ltp, \
                     tc.tile_pool(name=f"le{branch_id}", bufs=3) as lep, \
                     tc.tile_pool(name=f"ls{branch_id}", bufs=4) as lsp:
                    for h in range(NHL):
                        pl = lvp.tile([128, CL], f32, tag="pl")
                        kview = kraw[:].rearrange("p (b r) -> p b r", r=CL)
                        rhs_all = kview[:, :, h * 64:h * 64 + 64]
                        nc.tensor.matmul(
                            pl[:], dmat[:, h * 256:h * 256 + 128],
                            rhs_all, start=True, stop=False)
                        rhs_prev = kview[:, 0:7, h * 64:h * 64 + 64]
                        nc.tensor.matmul(
                            pl[:, 64:512], dmat[:, h * 256 + 128:h * 256 + 256],
                            rhs_prev, start=False, stop=True)
                        lraw = lep.tile([128, CL], f32, tag="lraw")
                        nc.scalar.copy(lraw[:], pl[:])
                        sq = lep.tile([128, CL], f32, tag="lsq")
                        nc.vector.tensor_tensor(sq[:], lraw[:], lraw[:],
                                                ALU.mult)
                        ssq = lsp.tile([128, 8], f32, tag="lssq")
                        nc.vector.reduce_sum(
                            ssq[:], sq[:].rearrange("p (b d) -> p b d", d=64),
                            axis=AXL.X)
                        lnv = lsp.tile([128, 8], f32, tag="llnv")
                        nc.scalar.activation(lnv[:], ssq[:], AF.Ln)
                        rn = lsp.tile([128, 8], f32, tag="lrn")
                        nc.scalar.activation(rn[:], lnv[:], AF.Exp, scale=-0.5)
                        rns = lsp.tile([128, 8], f32, tag="lrns")
                        nc.vector.tensor_scalar_mul(rns[:], rn[:],
                                                    svec[:, h:h + 1])
                        kfeat = lep.tile([128, CL], f32r, tag="kfeat")
                        rb = rns[:].unsqueeze(2).broadcast_to((128, 8, 64))
                        nc.vector.tensor_tensor(
                            kfeat[:].rearrange("p (b d) -> p b d", d=64),
                            lraw[:].rearrange("p (b d) -> p b d", d=64),
                            rb, ALU.mult)
                        # transpose 8 blocks of (128,64) -> (64,128)
                        pbase = (h % 2) * 64
                        fbase = (h // 2) * 1024
                        for half in range(2):
                            ptr = ltp.tile([64, 512], f32, tag="ktr")
                            for q in range(4):
                                blk = half * 4 + q
                                nc.tensor.transpose(
                                    ptr[:, q * 128:(q + 1) * 128]
                                    .bitcast(f32r),
                                    kfeat[:, blk * 64:(blk + 1) * 64],
                                    ident[:])
                            nc.scalar.copy(
                                ktall[pbase:pbase + 64,
                                      fbase + half * 512:fbase + half * 512
                                      + 512],
                                ptr[:])

                # ---- attention ----
                ytall = wp.tile([128, 4096], f32r, tag="ytall")
                with tc.tile_pool(name=f"as{branch_id}", bufs=3,
                                  space="PSUM") as asp, \
                     tc.tile_pool(name=f"ay{branch_id}", bufs=2,
                                  space="PSUM") as ayp, \
                     tc.tile_pool(name=f"ab{branch_id}", bufs=1,
                                  space="PSUM") as abp, \
                     tc.tile_pool(name=f"at{branch_id}", bufs=3) as atp, \
                     tc.tile_pool(name=f"ar{branch_id}", bufs=2) as arp:
                    for h in range(NHL):
                        pbase = (h % 2) * 64
                        fbase = (h // 2) * 1024
                        kt_h = ktall[pbase:pbase + 64, fbase:fbase + 1024]
                        for qc in range(2):
                            py = ayp.tile([66, 512], f32, tag="py")
                            njb = 4 if (is_ctx and qc == 0) else 8
                            for jb in range(njb):
                                ps = asp.tile([128, 512], f32, tag="ps")
                                if is_ctx:
                                    lhs_sc = kt_h[:, jb * 128:(jb + 1) * 128]
                                else:
                                    lhs_sc = pktall[pbase:pbase + 64,
                                                    fbase + jb * 128:
                                                    fbase + (jb + 1) * 128]
                                nc.tensor.matmul(
                                    ps[:], lhs_sc,
                                    kt_h[:, qc * 512:(qc + 1) * 512],
                                    start=True, stop=True)
                                att = atp.tile([128, 512], f32r, tag="att")
                                r = jb - qc * 4
                                if is_ctx and r >= 0:
                                    araw = atp.tile([128, 512], f32,
                                                    tag="araw")
                                    nc.scalar.activation(araw[:], ps[:],
                                                         AF.Exp)
                                    nc.vector.tensor_tensor(
                                        att[:], araw[:],
                                        maskt[:, r * 512:(r + 1) * 512],
                                        ALU.mult)
                                else:
                                    nc.scalar.activation(att[:], ps[:],
                                                         AF.Exp)
                                if is_ctx:
                                    lhs_v = vnorm[:, jb * 528 + h * 66:
                                                  jb * 528 + (h + 1) * 66]
                                else:
                                    lhs_v = pvo[:, h * 528 + jb * 66:
                                                h * 528 + (jb + 1) * 66]
                                nc.tensor.matmul(py[:], lhs_v, att[:],
                                                 start=(jb == 0),
                                                 stop=(jb == njb - 1))
                            grec = arp.tile([1, 512], f32r, tag="grec")
                            with nc.allow_low_precision(
                                    reason="f32r storage is 4-byte"):
                                nc.vector.reciprocal(grec[:], py[64:65, :])
                            pb = abp.tile([64, 512], f32, tag="pb")
                            nc.tensor.matmul(pb[:], ones1[:], grec[:],
                                             start=True, stop=True)
                            bcs = atp.tile([64, 512], f32, tag="bcs")
                            nc.scalar.copy(bcs[:], pb[:])
                            nc.vector.tensor_tensor(
                                ytall[pbase:pbase + 64,
                                      fbase + qc * 512:fbase + qc * 512 + 512],
                                py[0:64, :], bcs[:], ALU.mult)
                        if is_ctx:
                            # zero out the t=0 column (query 0 has no keys)
                            nc.sync.dma_start(
                                ytall[pbase:pbase + 64, fbase:fbase + 1],
                                zcol[0:64, :])

                # ---- c_proj -> all-reduce -> residual into X ----
                with tc.tile_pool(name=f"cp{branch_id}", bufs=2,
                                  space="PSUM") as cpp, \
                     tc.tile_pool(name=f"cw{branch_id}", bufs=2) as cwp, \
                     tc.tile_pool(name=f"cs{branch_id}", bufs=2) as csp:
                    cpw = [None] * 4
                    for cb in range(4):
                        cpw[cb] = cwp.tile([128, 1024], f32r, name=f"cpw{cb}", tag=f"cpw{cb}")
                        nc.sync.dma_start(
                            cpw[cb][:],
                            cp_in[cb * 128:(cb + 1) * 128, :].bitcast(f32r))
                    for tb in range(NTB):
                        stage = csp.tile([128, 1024], f32, tag="cstage")
                        for co in range(2):
                            pc = cpp.tile([128, 512], f32, tag="pc")
                            for cb in range(4):
                                nc.tensor.matmul(
                                    pc[:],
                                    ytall[:, cb * 1024 + tb * 128:
                                          cb * 1024 + tb * 128 + 128],
                                    cpw[cb][:, co * 512:(co + 1) * 512],
                                    start=(cb == 0), stop=(cb == 3))
                            nc.scalar.copy(stage[:, co * 512:(co + 1) * 512],
                                           pc[:])
                        nc.sync.dma_start(
                            cc_in_t[tb * 128:(tb + 1) * 128, :], stage[:])
                    nc.gpsimd.collective_compute(
                        "AllReduce", ALU.add, replica_groups=GROUPS,
                        ins=[cc_in_t[:]], outs=[cc_out_t[:]])
                    for tb in range(NTB):
                        back = csp.tile([128, 1024], f32, tag="cback")
                        nc.sync.dma_start(
                            back[:], cc_out_t[tb * 128:(tb + 1) * 128, :])
                        nc.vector.tensor_tensor(
                            X[:, tb * 1024:(tb + 1) * 1024],
                            X[:, tb * 1024:(tb + 1) * 1024].bitcast(f32),
                            back[:], ALU.add)

            # -------- context branch --------
            pktall = None
            branch(0, wka_in, wsb["wka"], dmata_in, sveca, cc1_in, cc1_out,
                   wv_in=wva_in, wvsum_bc=wsb["wva"], cp_in=cpa_in)

            # load persistent-memory keys into the vnorm slot (context-only)
            pktall = wp.tile([128, 4224], f32r, tag="vnorm")
            nc.sync.dma_start(pktall[:, 0:4096], pkt_in[:].bitcast(f32r))

            # -------- persistent branch --------
            branch(1, wkm_in, wsb["wkm"], dmatm_in, svecm, cc2_in, cc2_out,
                   cp_in=cpm_in)

            # X now holds the final output
            nc.sync.dma_start(
                out_t[:].bitcast(f32r).rearrange("(b p) c -> p b c", p=128),
                X[:].rearrange("p (b c) -> p b c", b=NTB))

    return nc


_prog_cache = {}


def _get_program():
    if "nc" not in _prog_cache:
        _prog_cache["nc"] = _build_program()
    return _prog_cache["nc"]


def _host_prep(inputs):
    """Build the 8 per-core input maps from the full-problem inputs."""
    x = np.asarray(inputs["x"], np.float32)
    ln1 = np.asarray(inputs["ln1_w"], np.float32)
    ln2 = np.asarray(inputs["ln2_w"], np.float32)
    Wk_a = np.asarray(inputs["Wk_a"], np.float32)
    Wv_a = np.asarray(inputs["Wv_a"], np.float32)
    cproj_a = np.asarray(inputs["cproj_a"], np.float32)
    beta_a = np.asarray(inputs["beta_a"], np.float32).reshape(NH)
    kscale_a = np.asarray(inputs["kscale_a"], np.float32).reshape(NH)
    vcoef = np.asarray(inputs["vcoef"], np.float32).reshape(NH)
    vscale = np.asarray(inputs["vscale"], np.float32).reshape(NH)
    Wk_m = np.asarray(inputs["Wk_m"], np.float32)
    beta_m = np.asarray(inputs["beta_m"], np.float32).reshape(NH)
    kscale_m = np.asarray(inputs["kscale_m"], np.float32).reshape(NH)
    Pk = np.asarray(inputs["Pk"], np.float32)
    Pv = np.asarray(inputs["Pv"], np.float32)
    out_scale = np.asarray(inputs["out_scale"], np.float32).reshape(NH)
    cproj_m = np.asarray(inputs["cproj_m"], np.float32)

    J, I = np.meshgrid(np.arange(128), np.arange(128), indexing="ij")

    def dmats(beta, heads):
        out = np.zeros((128, NHL * 256), np.float32)
        for i, h in enumerate(heads):
            b = abs(float(beta[h])) * EXP_SCALING
            out[:, i * 256:i * 256 + 128] = np.where(
                I >= J, np.exp(-(I - J) * b), 0.0)
            out[:, i * 256 + 128:i * 256 + 256] = np.exp(-((I + 128) - J) * b)
        return out

    # context diagonal masks: mask_r[jl, ql] = 1 if jl + r*128 < ql
    maskt = np.zeros((128, 2048), np.float32)
    jl = np.arange(128)[:, None]
    ql = np.arange(512)[None, :]
    for r in range(4):
        maskt[:, r * 512:(r + 1) * 512] = (jl + r * 128 < ql)

    vpad = np.zeros((128, 16), np.float32)
    vpad[:, 0::2] = 1.0

    base = {
        "ident": np.eye(128, dtype=np.float32),
        "maskt": maskt,
        "ones1": np.ones((1, 64), np.float32),
        "vpad": vpad,
        "zcol": np.zeros((128, 1), np.float32),
        "epsv": np.full((128, 1), 1e-5, np.float32),
        "zrow": np.zeros((1, 512), np.float32),
    }

    in_maps = []
    for c in range(N_CORES):
        b = c // 2
        hh = c % 2
        cols = slice(hh * CL, (hh + 1) * CL)
        heads = list(range(hh * NHL, hh * NHL + NHL))

        wka = (Wk_a * ln1[None, :])[cols].T.copy()      # (C, 512)
        wva = (Wv_a * ln1[None, :])[cols].T.copy()
        wkm = (Wk_m * ln2[None, :])[cols].T.copy()

        sva = np.exp(np.minimum(1.0 * EXP_SCALING * kscale_a[heads],
                                KSCALE_MAX))
        svm = np.exp(np.minimum(2.0 * EXP_SCALING * kscale_m[heads],
                                KSCALE_MAX))
        vs = np.exp(EXP_SCALING * vscale[heads])
        c1 = 1.0 - vcoef[heads]
        osc = np.exp(EXP_SCALING * out_scale[heads]) / Pk.shape[0]

        pkt = np.zeros((128, 4096), np.float32)
        pvo = np.zeros((128, NHL * 8 * 66), np.float32)
        for i, h in enumerate(heads):
            pb_ = (i % 2) * 64
            fb = (i // 2) * 1024
            pkt[pb_:pb_ + 64, fb:fb + 1024] = Pk[0, 0, h].T
            for pb2 in range(8):
                col = i * 528 + pb2 * 66
                pvo[:, col:col + 64] = Pv[0, 0, h, pb2 * 128:(pb2 + 1) * 128,
                                          :] * osc[i]
                pvo[:, col + 64] = 1.0
                pvo[:, col + 65] = 0.0

        m = dict(base)
        m.update({
            "x": np.ascontiguousarray(x[b]),
            "wka": np.ascontiguousarray(wka),
            "wva": np.ascontiguousarray(wva),
            "wkm": np.ascontiguousarray(wkm),
            "cpa": np.ascontiguousarray(cproj_a[:, cols].T),
            "cpm": np.ascontiguousarray(cproj_m[:, cols].T),
            "wkasb": np.broadcast_to(wka.sum(0), (128, CL)).copy(),
            "wvasb": np.broadcast_to(wva.sum(0), (128, CL)).copy(),
            "wkmsb": np.broadcast_to(wkm.sum(0), (128, CL)).copy(),
            "dmata": dmats(beta_a, heads),
            "dmatm": dmats(beta_m, heads),
            "sveca": np.broadcast_to(sva, (128, NHL)).copy(),
            "svecm": np.broadcast_to(svm, (128, NHL)).copy(),
            "coef1": np.broadcast_to(c1, (128, NHL)).copy(),
            "vs": np.broadcast_to(vs, (128, NHL)).copy(),
            "pkt": pkt,
            "pvo": pvo,
        })
        in_maps.append(m)
    return in_maps


def kernel(**inputs):
    nc = _get_program()
    in_maps = _host_prep(inputs)
    res = run_bass_kernel_spmd(nc, in_maps, list(range(N_CORES)))
    out = np.stack([res.results[2 * b]["out"] for b in range(B)], axis=0)
    return out.astype(np.float32)


def kernel_traced(**inputs):
    """Like kernel() but returns (out, BassKernelResults) with HW timing."""
    nc = _get_program()
    in_maps = _host_prep(inputs)
    res = run_bass_kernel_spmd(nc, in_maps, list(range(N_CORES)), trace=True)
    out = np.stack([res.results[2 * b]["out"] for b in range(B)], axis=0)
    return out.astype(np.float32), res


# revision 17
# speedup vs baseline: 1.1014x; 1.1014x over previous
"""Trainium2 Bass kernel for nn_Block_75840532513234 (dense transformer block).

Sharding: 8 cores; core c handles batch b = c//2 and head-half hh = c%2
(8 of 16 heads). The two cores of a pair all-reduce the c_proj partial sums
(row-sharded c_proj) after each of the two sub-blocks, so both hold the full
residual stream; the pair's even core's output is used.

Layout strategy (per core):
  - x, x' kept natural (t-part, c-free); LN stats per-partition.
  - LN mean/scale folded into projection evictions as a rank-1 correction
    ((x@W - mu*colsum(W)) * rstd), so x is transposed raw on the PE.
  - LeakyAvg = banded matmul with two 128x128 per-head decay matrices
    (exp decay below ~1e-28 beyond 128 steps for beta >= 0.5).
  - Attention computed fully transposed: scoresT = KT.T-block @ KT,
    attT = exp(scoresT) (softmax needs no max subtraction: |scores| <= ~10),
    YT = [V|1].T @ attT gives Y and the denominators in one accumulation.
  - ln1/ln2 weights folded into W_k/W_v host-side; kscale via svec;
    out_scale/PM_COUNT folded into Pv host-side; vscale into v-normalize.
"""
import sys
import math

sys.path.insert(0, '/opt/trn_rl_repo')

import numpy as np

# ---------------------------------------------------------------------------
# Patches for this container's walrus build: it allows only ONE sync-wait per
# instruction, while Tile attaches several (final drain; ldweights). Split the
# extras onto standalone single-wait EventSemaphore instructions.
# ---------------------------------------------------------------------------
import concourse.tile as tile
import concourse.bass as bass
from concourse import mybir
from concourse.vector_clock import ScopedClock

_ev_ctr = [0]


def _split_multi_waits(nc):
    for f in nc.m.functions:
        for bb in f.blocks:
            il = bb.instructions
            i = 0
            while i < len(il):
                inst = il[i]
                si = inst.sync_info
                if si is not None and si.on_wait and len(si.on_wait) > 1:
                    waits = list(si.on_wait)
                    si.on_wait.clear()
                    si.on_wait.append(waits[-1])
                    for w in waits[:-1]:
                        _ev_ctr[0] += 1
                        ev = mybir.InstEventSemaphore(
                            name=f"EVSPLIT-{_ev_ctr[0]}", ins=[], outs=[])
                        ev.engine = inst.engine
                        ev.sync_info = mybir.SyncInfo(on_wait=[], on_update=[])
                        ev.sync_info.on_wait.append(w)
                        il.insert(i, ev)
                        i += 1
                i += 1


def _patched_drain_and_barrier(self, tick_clock, wait_clock):
    nc = self.nc
    drain_inst = nc.sync.drain()
    wait_clock.add_sem_waits(
        drain_inst.ins, ScopedClock({None: tick_clock.global_clock}))
    nc.all_engine_barrier()
    popped = nc._tile_sem_poison_stack.pop()
    assert popped is self._sem_poison
    nc.clear_and_free_semaphores(list(self.sems.allocated().values()))
    nc.all_engine_barrier()


tile.TileContext._drain_and_barrier = _patched_drain_and_barrier

_orig_tile_exit = tile.TileContext.__exit__


def _patched_tile_exit(self, *a, **k):
    r = _orig_tile_exit(self, *a, **k)
    _split_multi_waits(self.nc)
    return r


tile.TileContext.__exit__ = _patched_tile_exit

# NTFF profile hook (trimmed image lacks antenv.axon_hooks).
import types as _types

if "antenv.axon_hooks" not in sys.modules:
    _m = _types.ModuleType("antenv.axon_hooks")
    _hook_store = [None]

    def _set_hook(h):
        _hook_store[0] = h

    def _get_hook():
        if _hook_store[0] is None:
            try:
                if '/root/.axon_site' not in sys.path:
                    sys.path.insert(0, '/root/.axon_site')
                from trn_agent_boot.trn_boot import _ntff_profile_via_ctypes
                _hook_store[0] = _ntff_profile_via_ctypes(
                    '/opt/axon/libaxon_pjrt.so')
            except Exception:
                return None
        return _hook_store[0]

    _m.set_axon_ntff_profile_hook = _set_hook
    _m.get_axon_ntff_profile_hook = _get_hook
    sys.modules["antenv.axon_hooks"] = _m
    import antenv as _antenv
    _antenv.axon_hooks = _m

from concourse.bass_utils import run_bass_kernel_spmd  # noqa: E402

# ---------------------------------------------------------------------------
# Problem constants (hardcoded per the grading contract)
# ---------------------------------------------------------------------------
B, T, C, NH = 4, 1024, 1024, 16
HS = C // NH              # 64
NHL = 8                   # heads per core
CL = NHL * HS             # 512 local channels
EXP_SCALING = 10.0
KSCALE_MAX = float(np.log(2.0 ** 16 - 1))
N_CORES = 8
GROUPS = [[0, 1], [2, 3], [4, 5], [6, 7]]

f32 = mybir.dt.float32
f32r = mybir.dt.float32r
AF = mybir.ActivationFunctionType
ALU = mybir.AluOpType
AXL = mybir.AxisListType

NTB = T // 128            # 8 t-blocks
NCB = C // 128            # 8 c-blocks


def _build_program():
    nc = bass.Bass(num_devices=N_CORES)

    # ---- I/O ----
    x_in = nc.dram_tensor("x", [T, C], f32, kind="ExternalInput")
    wka_in = nc.dram_tensor("wka", [C, CL], f32, kind="ExternalInput")
    wva_in = nc.dram_tensor("wva", [C, CL], f32, kind="ExternalInput")
    wkm_in = nc.dram_tensor("wkm", [C, CL], f32, kind="ExternalInput")
    cpa_in = nc.dram_tensor("cpa", [CL, C], f32, kind="ExternalInput")
    cpm_in = nc.dram_tensor("cpm", [CL, C], f32, kind="ExternalInput")
    wkasb_in = nc.dram_tensor("wkasb", [128, CL], f32, kind="ExternalInput")
    wvasb_in = nc.dram_tensor("wvasb", [128, CL], f32, kind="ExternalInput")
    wkmsb_in = nc.dram_tensor("wkmsb", [128, CL], f32, kind="ExternalInput")
    dmata_in = nc.dram_tensor("dmata", [128, NHL * 256], f32, kind="ExternalInput")
    dmatm_in = nc.dram_tensor("dmatm", [128, NHL * 256], f32, kind="ExternalInput")
    sveca_in = nc.dram_tensor("sveca", [128, NHL], f32, kind="ExternalInput")
    svecm_in = nc.dram_tensor("svecm", [128, NHL], f32, kind="ExternalInput")
    coef1_in = nc.dram_tensor("coef1", [128, NHL], f32, kind="ExternalInput")
    vs_in = nc.dram_tensor("vs", [128, NHL], f32, kind="ExternalInput")
    pkt_in = nc.dram_tensor("pkt", [128, 4096], f32, kind="ExternalInput")
    pvo_in = nc.dram_tensor("pvo", [128, NHL * 8 * 66], f32, kind="ExternalInput")
    ident_in = nc.dram_tensor("ident", [128, 128], f32, kind="ExternalInput")
    maskt_in = nc.dram_tensor("maskt", [128, 4 * 512], f32, kind="ExternalInput")
    ones1_in = nc.dram_tensor("ones1", [1, 64], f32, kind="ExternalInput")
    vpad_in = nc.dram_tensor("vpad", [128, 16], f32, kind="ExternalInput")
    zcol_in = nc.dram_tensor("zcol", [128, 1], f32, kind="ExternalInput")
    epsv_in = nc.dram_tensor("epsv", [128, 1], f32, kind="ExternalInput")
    zrow_in = nc.dram_tensor("zrow", [1, 512], f32, kind="ExternalInput")

    xp_out = nc.dram_tensor("xp", [T, C], f32, kind="ExternalOutput")
    pm_out = nc.dram_tensor("pm", [512, C], f32, kind="ExternalOutput")

    cc1_in = nc.dram_tensor("cc1_in", [T, C], f32)
    cc1_out = nc.dram_tensor("cc1_out", [T, C], f32)
    cc2_in = nc.dram_tensor("cc2_in", [T, C], f32)
    cc2_out = nc.dram_tensor("cc2_out", [T, C], f32)

    with tile.TileContext(nc) as tc:
        # ---------------- persistent pools ----------------
        with tc.tile_pool(name="persist", bufs=1) as pp, \
             tc.tile_pool(name="work", bufs=1) as wp:
            # constants
            ident = pp.tile([128, 128], f32r, tag="ident")
            nc.sync.dma_start(ident[:], ident_in[:].bitcast(f32r))
            sveca = pp.tile([128, NHL], f32, tag="sveca")
            nc.sync.dma_start(sveca[:], sveca_in[:])
            svecm = pp.tile([128, NHL], f32, tag="svecm")
            nc.sync.dma_start(svecm[:], svecm_in[:])
            coef1 = pp.tile([128, NHL], f32, tag="coef1")
            nc.sync.dma_start(coef1[:], coef1_in[:])
            vsv = pp.tile([128, NHL], f32, tag="vsv")
            nc.sync.dma_start(vsv[:], vs_in[:])
            ones1 = pp.tile([1, 64], f32r, tag="ones1")
            nc.sync.dma_start(ones1[:], ones1_in[:].bitcast(f32r))
            vpad = pp.tile([128, 16], f32r, tag="vpad")
            nc.sync.dma_start(vpad[:], vpad_in[:].bitcast(f32r))
            zcol = pp.tile([128, 1], f32r, tag="zcol")
            nc.sync.dma_start(zcol[:], zcol_in[:].bitcast(f32r))
            epsv = pp.tile([128, 1], f32, tag="epsv")
            nc.sync.dma_start(epsv[:], epsv_in[:])
            maskt = pp.tile([128, 2048], f32, tag="maskt")
            nc.sync.dma_start(maskt[:], maskt_in[:])
            pvo = pp.tile([128, NHL * 8 * 66], f32r, tag="pvo")
            nc.sync.dma_start(pvo[:], pvo_in[:].bitcast(f32r))
            wsb = {}
            for nm, src in (("wka", wkasb_in), ("wva", wvasb_in),
                            ("wkm", wkmsb_in)):
                wsb[nm] = pp.tile([128, CL], f32, name=f"wsb_{nm}", tag=f"wsb_{nm}")
                nc.sync.dma_start(wsb[nm][:], src[:])

            # big persistent buffers (tags reused across the two branches)
            # X is f32r so PE can transpose it directly (rounds x once, ~1e-4)
            X = wp.tile([128, NTB * 1024], f32r, tag="X")        # x then out
            nc.sync.dma_start(
                X[:].rearrange("p (b c) -> p b c", b=NTB),
                x_in[:].bitcast(f32r).rearrange("(b p) c -> p b c", p=128))

            def branch(branch_id, wk_in, wsum_bc, dmat_in, svec, cc_in_t,
                       cc_out_t, wv_in=None, wvsum_bc=None, cp_in=None):
                """Emit one sub-block (context: with v; persistent: without).

                Returns nothing; accumulates residual into X in place.
                branch uses tiles tagged with shared tags so the second call
                reuses the first call's SBUF.
                """
                is_ctx = wv_in is not None

                # ---- LN stats ----
                negmu = []
                rstd = []
                with tc.tile_pool(name=f"st{branch_id}", bufs=2 * NTB) as sp, \
                     tc.tile_pool(name=f"sttmp{branch_id}", bufs=2) as stp:
                    for tb in range(NTB):
                        xs = X[:, tb * 1024:(tb + 1) * 1024].bitcast(f32)
                        s = stp.tile([128, 1], f32, tag="s")
                        nc.vector.reduce_sum(s[:], xs, axis=AXL.X)
                        sq = stp.tile([128, 1024], f32, tag="sq")
                        nc.vector.tensor_tensor(sq[:], xs, xs, ALU.mult)
                        ssq = stp.tile([128, 1], f32, tag="ssq")
                        nc.vector.reduce_sum(ssq[:], sq[:], axis=AXL.X)
                        nm = sp.tile([128, 1], f32, tag="negmu")
                        nc.vector.tensor_scalar_mul(nm[:], s[:], -1.0 / C)
                        mp = stp.tile([128, 1], f32, tag="mp")
                        nc.vector.tensor_scalar_mul(mp[:], s[:], 1.0 / C)
                        sn = stp.tile([128, 1], f32, tag="sn")
                        nc.vector.tensor_scalar_mul(sn[:], ssq[:], 1.0 / C)
                        var = stp.tile([128, 1], f32, tag="var")
                        nc.vector.scalar_tensor_tensor(
                            var[:], mp[:], nm[:], sn[:], ALU.mult, ALU.add)
                        lv = stp.tile([128, 1], f32, tag="lv")
                        nc.scalar.activation(lv[:], var[:], AF.Ln, bias=epsv[:])
                        rs = sp.tile([128, 1], f32, tag="rstd")
                        nc.scalar.activation(rs[:], lv[:], AF.Exp, scale=-0.5)
                        negmu.append(nm)
                        rstd.append(rs)

                    # ---- transpose + projections (grouped for PSUM) ----
                    kraw = wp.tile([128, NTB * CL], f32r, tag="kraw")
                    vraw = None
                    if is_ctx:
                        vraw = wp.tile([128, NTB * CL], f32, tag="vraw")
                    tgroups = [(0, 3), (3, 6), (6, 8)]
                    with tc.tile_pool(name=f"pj{branch_id}", bufs=1,
                                      space="PSUM") as pjp, \
                         tc.tile_pool(name=f"ptr{branch_id}", bufs=2,
                                      space="PSUM") as ptp, \
                         tc.tile_pool(name=f"w{branch_id}", bufs=4) as wpool, \
                         tc.tile_pool(name=f"xtc{branch_id}", bufs=2) as xtp, \
                         tc.tile_pool(name=f"ev{branch_id}", bufs=3) as evp:
                        for g0, g1 in tgroups:
                            gn = g1 - g0
                            psk = [pjp.tile([128, CL], f32,
                                            name=f"psk{g0}_{i}",
                                            tag=f"psk{i}")
                                   for i in range(gn)]
                            psv = [pjp.tile([128, CL], f32,
                                            name=f"psv{g0}_{i}",
                                            tag=f"psv{i}")
                                   for i in range(gn)] if is_ctx else None
                            for cb in range(NCB):
                                xtc = xtp.tile([128, gn * 128], f32r,
                                               tag="xtc")
                                for i, tb in enumerate(range(g0, g1)):
                                    ptr = ptp.tile([128, 128], f32, tag="ptr")
                                    nc.tensor.transpose(
                                        ptr[:].bitcast(f32r),
                                        X[:, tb * 1024 + cb * 128:
                                          tb * 1024 + cb * 128 + 128],
                                        ident[:])
                                    nc.scalar.copy(
                                        xtc[:, i * 128:(i + 1) * 128],
                                        ptr[:])
                                wk_c = wpool.tile([128, CL], f32r, tag="wk")
                                nc.sync.dma_start(
                                    wk_c[:],
                                    wk_in[cb * 128:(cb + 1) * 128, :]
                                    .bitcast(f32r))
                                wv_c = None
                                if is_ctx:
                                    wv_c = wpool.tile([128, CL], f32r,
                                                      tag="wv")
                                    nc.sync.dma_start(
                                        wv_c[:],
                                        wv_in[cb * 128:(cb + 1) * 128, :]
                                        .bitcast(f32r))
                                for i in range(gn):
                                    nc.tensor.matmul(
                                        psk[i][:],
                                        xtc[:, i * 128:(i + 1) * 128],
                                        wk_c[:], start=(cb == 0),
                                        stop=(cb == NCB - 1))
                                    if is_ctx:
                                        nc.tensor.matmul(
                                            psv[i][:],
                                            xtc[:, i * 128:(i + 1) * 128],
                                            wv_c[:], start=(cb == 0),
                                            stop=(cb == NCB - 1))
                            # evict with LN fold: (psum + negmu*wsum)*rstd
                            # kraw first: it gates the PE (LeakyAvg)
                            for i, tb in enumerate(range(g0, g1)):
                                tmp = evp.tile([128, CL], f32, tag="evt")
                                nc.vector.scalar_tensor_tensor(
                                    tmp[:], wsum_bc[:], negmu[tb][:],
                                    psk[i][:], ALU.mult, ALU.add)
                                nc.vector.tensor_scalar_mul(
                                    kraw[:, tb * CL:(tb + 1) * CL],
                                    tmp[:], rstd[tb][:])
                            for i, tb in enumerate(range(g0, g1)):
                                if is_ctx:
                                    tmp2 = evp.tile([128, CL], f32,
                                                    tag="evt2")
                                    nc.vector.scalar_tensor_tensor(
                                        tmp2[:], wvsum_bc[:], negmu[tb][:],
                                        psv[i][:], ALU.mult, ALU.add)
                                    nc.vector.tensor_scalar_mul(
                                        vraw[:, tb * CL:(tb + 1) * CL],
                                        tmp2[:], rstd[tb][:])

                # ---- v path (context only) ----
                vnorm = None
                if is_ctx:
                    vnorm = wp.tile([128, NTB * 528], f32r, tag="vnorm")
                    with tc.tile_pool(name="vtmp", bufs=1) as vtp, \
                         tc.tile_pool(name="vst", bufs=1) as vsp:
                        FS = NTB * CL
                        vsh = vtp.tile([128, FS], f32, tag="vsh")
                        # v_shift via DMA: partition +1, wrap to next t-block
                        nc.sync.dma_start(vsh[0:127, :], vraw[1:128, :])
                        nc.sync.dma_start(vsh[127:128, 0:FS - CL],
                                          vraw[0:1, CL:FS])
                        nc.sync.dma_start(vsh[127:128, FS - CL:FS],
                                          zrow_in[:])
                        # diff/vmix computed in place in vsh
                        nc.vector.tensor_tensor(vsh[:], vsh[:], vraw[:],
                                                ALU.subtract)
                        c_b = coef1[:].unsqueeze(1).unsqueeze(3).broadcast_to(
                            (128, NTB, NHL, HS))
                        nc.vector.tensor_tensor(
                            vsh[:].rearrange("p (b h d) -> p b h d", b=NTB,
                                             h=NHL),
                            vsh[:].rearrange("p (b h d) -> p b h d", b=NTB,
                                             h=NHL),
                            c_b, ALU.mult)
                        nc.vector.tensor_tensor(vsh[:], vsh[:], vraw[:],
                                                ALU.add)
                        vmix = vsh
                        sq = vtp.tile([128, FS], f32, tag="vsq")
                        nc.vector.tensor_tensor(sq[:], vmix[:], vmix[:],
                                                ALU.mult)
                        ssq = vsp.tile([128, NTB * NHL], f32, tag="vssq")
                        nc.vector.reduce_sum(
                            ssq[:], sq[:].rearrange("p (g d) -> p g d", d=HS),
                            axis=AXL.X)
                        lnv = vsp.tile([128, NTB * NHL], f32, tag="vlnv")
                        nc.scalar.activation(lnv[:], ssq[:], AF.Ln)
                        rn = vsp.tile([128, NTB * NHL], f32, tag="vrn")
                        nc.scalar.activation(rn[:], lnv[:], AF.Exp,
                                             scale=-0.5)
                        rns = vsp.tile([128, NTB * NHL], f32, tag="vrns")
                        vs_b = vsv[:].unsqueeze(1).broadcast_to(
                            (128, NTB, NHL))
                        nc.vector.tensor_tensor(
                            rns[:].rearrange("p (b h) -> p b h", b=NTB),
                            rn[:].rearrange("p (b h) -> p b h", b=NTB),
                            vs_b, ALU.mult)
                        rns_b = rns[:].rearrange(
                            "p (b h) -> p b h", b=NTB).unsqueeze(
                            3).broadcast_to((128, NTB, NHL, HS))
                        nc.vector.tensor_tensor(
                            vnorm[:].rearrange("p (b h c) -> p b h c", b=NTB,
                                               h=NHL)[:, :, :, 0:64],
                            vmix[:].rearrange("p (b h d) -> p b h d", b=NTB,
                                              h=NHL),
                            rns_b, ALU.mult)
                        # ones/zeros pad columns for all t-blocks at once
                        vp_b = vpad[:].rearrange("p (h t) -> p h t", h=NHL)
                        for tb in range(NTB):
                            nc.sync.dma_start(
                                vnorm[:, tb * 528:(tb + 1) * 528].rearrange(
                                    "p (h c) -> p h c", h=NHL)[:, :, 64:66],
                                vp_b)

                # ---- LeakyAvg + normalize + transpose -> ktall ----
                dmat = wp.tile([128, NHL * 256], f32r, tag="dmat")
                nc.sync.dma_start(dmat[:], dmat_in[:].bitcast(f32r))
                ktall = wp.tile([128, 4096], f32r, tag="ktall")
                with tc.tile_pool(name=f"lv{branch_id}", bufs=2,
                                  space="PSUM") as lvp, \
                     tc.tile_pool(name=f"ltr{branch_id}", bufs=2,
                                  space="PSUM") as ltp, \
                     tc.tile_pool(name=f"le{branch_id}", bufs=3) as lep, \
                     tc.tile_pool(name=f"ls{branch_id}", bufs=4) as lsp:
                    for h in range(NHL):
                        pl = lvp.tile([128, CL], f32, tag="pl")
                        kview = kraw[:].rearrange("p (b r) -> p b r", r=CL)
                        rhs_all = kview[:, :, h * 64:h * 64 + 64]
                        nc.tensor.matmul(
                            pl[:], dmat[:, h * 256:h * 256 + 128],
                            rhs_all, start=True, stop=False)
                        rhs_prev = kview[:, 0:7, h * 64:h * 64 + 64]
                        nc.tensor.matmul(
                            pl[:, 64:512], dmat[:, h * 256 + 128:h * 256 + 256],
                            rhs_prev, start=False, stop=True)
                        lraw = lep.tile([128, CL], f32, tag="lraw")
                        nc.scalar.copy(lraw[:], pl[:])
                        sq = lep.tile([128, CL], f32, tag="lsq")
                        nc.vector.tensor_tensor(sq[:], lraw[:], lraw[:],
                                                ALU.mult)
                        ssq = lsp.tile([128, 8], f32, tag="lssq")
                        nc.vector.reduce_sum(
                            ssq[:], sq[:].rearrange("p (b d) -> p b d", d=64),
                            axis=AXL.X)
                        lnv = lsp.tile([128, 8], f32, tag="llnv")
                        nc.scalar.activation(lnv[:], ssq[:], AF.Ln)
                        rn = lsp.tile([128, 8], f32, tag="lrn")
                        nc.scalar.activation(rn[:], lnv[:], AF.Exp, scale=-0.5)
                        rns = lsp.tile([128, 8], f32, tag="lrns")
                        nc.vector.tensor_scalar_mul(rns[:], rn[:],
                                                    svec[:, h:h + 1])
                        kfeat = lep.tile([128, CL], f32r, tag="kfeat")
                        rb = rns[:].unsqueeze(2).broadcast_to((128, 8, 64))
                        nc.vector.tensor_tensor(
                            kfeat[:].rearrange("p (b d) -> p b d", d=64),
                            lraw[:].rearrange("p (b d) -> p b d", d=64),
                            rb, ALU.mult)
                        # transpose 8 blocks of (128,64) -> (64,128)
                        pbase = (h % 2) * 64
                        fbase = (h // 2) * 1024
                        for half in range(2):
                            ptr = ltp.tile([64, 512], f32, tag="ktr")
                            for q in range(4):
                                blk = half * 4 + q
                                nc.tensor.transpose(
                                    ptr[:, q * 128:(q + 1) * 128]
                                    .bitcast(f32r),
                                    kfeat[:, blk * 64:(blk + 1) * 64],
                                    ident[:])
                            nc.scalar.copy(
                                ktall[pbase:pbase + 64,
                                      fbase + half * 512:fbase + half * 512
                                      + 512],
                                ptr[:])

                # ---- attention ----
                ytall = wp.tile([128, 4096], f32r, tag="ytall")
                with tc.tile_pool(name=f"as{branch_id}", bufs=3,
                                  space="PSUM") as asp, \
                     tc.tile_pool(name=f"ay{branch_id}", bufs=2,
                                  space="PSUM") as ayp, \
                     tc.tile_pool(name=f"ab{branch_id}", bufs=1,
                                  space="PSUM") as abp, \
                     tc.tile_pool(name=f"at{branch_id}", bufs=3) as atp, \
                     tc.tile_pool(name=f"ar{branch_id}", bufs=2) as arp:
                    for h in range(NHL):
                        pbase = (h % 2) * 64
                        fbase = (h // 2) * 1024
                        kt_h = ktall[pbase:pbase + 64, fbase:fbase + 1024]
                        for qc in range(2):
                            py = ayp.tile([66, 512], f32, tag="py")
                            njb = 4 if (is_ctx and qc == 0) else 8
                            for jb in range(njb):
                                ps = asp.tile([128, 512], f32, tag="ps")
                                if is_ctx:
                                    lhs_sc = kt_h[:, jb * 128:(jb + 1) * 128]
                                else:
                                    lhs_sc = pktall[pbase:pbase + 64,
                                                    fbase + jb * 128:
                                                    fbase + (jb + 1) * 128]
                                nc.tensor.matmul(
                                    ps[:], lhs_sc,
                                    kt_h[:, qc * 512:(qc + 1) * 512],
                                    start=True, stop=True)
                                att = atp.tile([128, 512], f32r, tag="att")
                                r = jb - qc * 4
                                if is_ctx and r >= 0:
                                    araw = atp.tile([128, 512], f32,
                                                    tag="araw")
                                    nc.scalar.activation(araw[:], ps[:],
                                                         AF.Exp)
                                    nc.vector.tensor_tensor(
                                        att[:], araw[:],
                                        maskt[:, r * 512:(r + 1) * 512],
                                        ALU.mult)
                                else:
                                    nc.scalar.activation(att[:], ps[:],
                                                         AF.Exp)
                                if is_ctx:
                                    lhs_v = vnorm[:, jb * 528 + h * 66:
                                                  jb * 528 + (h + 1) * 66]
                                else:
                                    lhs_v = pvo[:, h * 528 + jb * 66:
                                                h * 528 + (jb + 1) * 66]
                                nc.tensor.matmul(py[:], lhs_v, att[:],
                                                 start=(jb == 0),
                                                 stop=(jb == njb - 1))
                            # 1/denom = exp(-ln(denom)) on ScalarE (2 ULP;
                            # stays in the natural_log_exp table set)
                            lrow = arp.tile([1, 512], f32, tag="lrow")
                            nc.scalar.activation(lrow[:], py[64:65, :], AF.Ln)
                            rrow = arp.tile([1, 512], f32r, tag="rrow")
                            nc.scalar.activation(rrow[:], lrow[:], AF.Exp,
                                                 scale=-1.0)
                            pb = abp.tile([64, 512], f32, tag="pb")
                            nc.tensor.matmul(pb[:], ones1[:], rrow[:],
                                             start=True, stop=True)
                            bcs = atp.tile([64, 512], f32, tag="bcs")
                            nc.scalar.copy(bcs[:], pb[:])
                            nc.vector.tensor_tensor(
                                ytall[pbase:pbase + 64,
                                      fbase + qc * 512:fbase + qc * 512 + 512],
                                py[0:64, :], bcs[:], ALU.mult)
                        if is_ctx:
                            # zero out the t=0 column (query 0 has no keys)
                            nc.sync.dma_start(
                                ytall[pbase:pbase + 64, fbase:fbase + 1],
                                zcol[0:64, :])

                # ---- c_proj -> collective -> residual into X ----
                # context: chunked AllReduce (both cores need full x').
                # persistent: ReduceScatter; each core finishes only its
                # half of the output rows (host stitches halves).
                with tc.tile_pool(name=f"cp{branch_id}", bufs=2,
                                  space="PSUM") as cpp, \
                     tc.tile_pool(name=f"cw{branch_id}", bufs=2) as cwp, \
                     tc.tile_pool(name=f"cs{branch_id}", bufs=2) as csp:
                    cpw = [None] * 4
                    for cb in range(4):
                        cpw[cb] = cwp.tile([128, 1024], f32r, name=f"cpw{cb}", tag=f"cpw{cb}")
                        nc.sync.dma_start(
                            cpw[cb][:],
                            cp_in[cb * 128:(cb + 1) * 128, :].bitcast(f32r))
                    for tb in range(NTB):
                        stage = csp.tile([128, 1024], f32, tag="cstage")
                        for co in range(2):
                            pc = cpp.tile([128, 512], f32, tag="pc")
                            for cb in range(4):
                                nc.tensor.matmul(
                                    pc[:],
                                    ytall[:, cb * 1024 + tb * 128:
                                          cb * 1024 + tb * 128 + 128],
                                    cpw[cb][:, co * 512:(co + 1) * 512],
                                    start=(cb == 0), stop=(cb == 3))
                            nc.scalar.copy(stage[:, co * 512:(co + 1) * 512],
                                           pc[:])
                        nc.sync.dma_start(
                            cc_in_t[tb * 128:(tb + 1) * 128, :], stage[:])
                        if is_ctx and tb in (3, NTB - 1):
                            # half-chunk AllReduce as soon as rows are staged
                            lo = 0 if tb == 3 else 512
                            nc.gpsimd.collective_compute(
                                "AllReduce", ALU.add, replica_groups=GROUPS,
                                ins=[cc_in_t[lo:lo + 512, :]],
                                outs=[cc_out_t[lo:lo + 512, :]])
                    if is_ctx:
                        for tb in range(NTB):
                            back = csp.tile([128, 1024], f32, tag="cback")
                            nc.sync.dma_start(
                                back[:], cc_out_t[tb * 128:(tb + 1) * 128, :])
                            nc.vector.tensor_tensor(
                                X[:, tb * 1024:(tb + 1) * 1024],
                                X[:, tb * 1024:(tb + 1) * 1024].bitcast(f32),
                                back[:], ALU.add)
                    else:
                        # each core gets its pair-rank's half of the summed
                        # pm; host adds it onto the matching half of x'.
                        nc.gpsimd.collective_compute(
                            "ReduceScatter", ALU.add, replica_groups=GROUPS,
                            ins=[cc_in_t[:]], outs=[cc_out_t[0:512, :]])
                        for i in range(4):
                            back = csp.tile([128, 1024], f32, tag="cback")
                            nc.sync.dma_start(
                                back[:], cc_out_t[i * 128:(i + 1) * 128, :])
                            nc.sync.dma_start(
                                pm_out[i * 128:(i + 1) * 128, :], back[:])

            # -------- context branch --------
            pktall = None
            branch(0, wka_in, wsb["wka"], dmata_in, sveca, cc1_in, cc1_out,
                   wv_in=wva_in, wvsum_bc=wsb["wva"], cp_in=cpa_in)

            # x' is final except for the pm half-add done on host
            nc.sync.dma_start(
                xp_out[:].bitcast(f32r).rearrange("(b p) c -> p b c", p=128),
                X[:].rearrange("p (b c) -> p b c", b=NTB))

            # load persistent-memory keys into the vnorm slot (context-only)
            pktall = wp.tile([128, 4224], f32r, tag="vnorm")
            nc.sync.dma_start(pktall[:, 0:4096], pkt_in[:].bitcast(f32r))

            # -------- persistent branch --------
            branch(1, wkm_in, wsb["wkm"], dmatm_in, svecm, cc2_in, cc2_out,
                   cp_in=cpm_in)



    return nc


_prog_cache = {}


def _get_program():
    if "nc" not in _prog_cache:
        _prog_cache["nc"] = _build_program()
    return _prog_cache["nc"]


def _host_prep(inputs):
    """Build the 8 per-core input maps from the full-problem inputs."""
    x = np.asarray(inputs["x"], np.float32)
    ln1 = np.asarray(inputs["ln1_w"], np.float32)
    ln2 = np.asarray(inputs["ln2_w"], np.float32)
    Wk_a = np.asarray(inputs["Wk_a"], np.float32)
    Wv_a = np.asarray(inputs["Wv_a"], np.float32)
    cproj_a = np.asarray(inputs["cproj_a"], np.float32)
    beta_a = np.asarray(inputs["beta_a"], np.float32).reshape(NH)
    kscale_a = np.asarray(inputs["kscale_a"], np.float32).reshape(NH)
    vcoef = np.asarray(inputs["vcoef"], np.float32).reshape(NH)
    vscale = np.asarray(inputs["vscale"], np.float32).reshape(NH)
    Wk_m = np.asarray(inputs["Wk_m"], np.float32)
    beta_m = np.asarray(inputs["beta_m"], np.float32).reshape(NH)
    kscale_m = np.asarray(inputs["kscale_m"], np.float32).reshape(NH)
    Pk = np.asarray(inputs["Pk"], np.float32)
    Pv = np.asarray(inputs["Pv"], np.float32)
    out_scale = np.asarray(inputs["out_scale"], np.float32).reshape(NH)
    cproj_m = np.asarray(inputs["cproj_m"], np.float32)

    J, I = np.meshgrid(np.arange(128), np.arange(128), indexing="ij")

    def dmats(beta, heads):
        out = np.zeros((128, NHL * 256), np.float32)
        for i, h in enumerate(heads):
            b = abs(float(beta[h])) * EXP_SCALING
            out[:, i * 256:i * 256 + 128] = np.where(
                I >= J, np.exp(-(I - J) * b), 0.0)
            out[:, i * 256 + 128:i * 256 + 256] = np.exp(-((I + 128) - J) * b)
        return out

    # context diagonal masks: mask_r[jl, ql] = 1 if jl + r*128 < ql
    maskt = np.zeros((128, 2048), np.float32)
    jl = np.arange(128)[:, None]
    ql = np.arange(512)[None, :]
    for r in range(4):
        maskt[:, r * 512:(r + 1) * 512] = (jl + r * 128 < ql)

    vpad = np.zeros((128, 16), np.float32)
    vpad[:, 0::2] = 1.0

    base = {
        "ident": np.eye(128, dtype=np.float32),
        "maskt": maskt,
        "ones1": np.ones((1, 64), np.float32),
        "vpad": vpad,
        "zcol": np.zeros((128, 1), np.float32),
        "epsv": np.full((128, 1), 1e-5, np.float32),
        "zrow": np.zeros((1, 512), np.float32),
    }

    in_maps = []
    for c in range(N_CORES):
        b = c // 2
        hh = c % 2
        cols = slice(hh * CL, (hh + 1) * CL)
        heads = list(range(hh * NHL, hh * NHL + NHL))

        wka = (Wk_a * ln1[None, :])[cols].T.copy()      # (C, 512)
        wva = (Wv_a * ln1[None, :])[cols].T.copy()
        wkm = (Wk_m * ln2[None, :])[cols].T.copy()

        sva = np.exp(np.minimum(1.0 * EXP_SCALING * kscale_a[heads],
                                KSCALE_MAX))
        svm = np.exp(np.minimum(2.0 * EXP_SCALING * kscale_m[heads],
                                KSCALE_MAX))
        vs = np.exp(EXP_SCALING * vscale[heads])
        c1 = 1.0 - vcoef[heads]
        osc = np.exp(EXP_SCALING * out_scale[heads]) / Pk.shape[0]

        pkt = np.zeros((128, 4096), np.float32)
        pvo = np.zeros((128, NHL * 8 * 66), np.float32)
        for i, h in enumerate(heads):
            pb_ = (i % 2) * 64
            fb = (i // 2) * 1024
            pkt[pb_:pb_ + 64, fb:fb + 1024] = Pk[0, 0, h].T
            for pb2 in range(8):
                col = i * 528 + pb2 * 66
                pvo[:, col:col + 64] = Pv[0, 0, h, pb2 * 128:(pb2 + 1) * 128,
                                          :] * osc[i]
                pvo[:, col + 64] = 1.0
                pvo[:, col + 65] = 0.0

        m = dict(base)
        m.update({
            "x": np.ascontiguousarray(x[b]),
            "wka": np.ascontiguousarray(wka),
            "wva": np.ascontiguousarray(wva),
            "wkm": np.ascontiguousarray(wkm),
            "cpa": np.ascontiguousarray(cproj_a[:, cols].T),
            "cpm": np.ascontiguousarray(cproj_m[:, cols].T),
            "wkasb": np.broadcast_to(wka.sum(0), (128, CL)).copy(),
            "wvasb": np.broadcast_to(wva.sum(0), (128, CL)).copy(),
            "wkmsb": np.broadcast_to(wkm.sum(0), (128, CL)).copy(),
            "dmata": dmats(beta_a, heads),
            "dmatm": dmats(beta_m, heads),
            "sveca": np.broadcast_to(sva, (128, NHL)).copy(),
            "svecm": np.broadcast_to(svm, (128, NHL)).copy(),
            "coef1": np.broadcast_to(c1, (128, NHL)).copy(),
            "vs": np.broadcast_to(vs, (128, NHL)).copy(),
            "pkt": pkt,
            "pvo": pvo,
        })
        in_maps.append(m)
    return in_maps


def _assemble(res):
    out = np.empty((B, T, C), np.float32)
    for b in range(B):
        xp = res.results[2 * b]["xp"]
        out[b] = xp
        out[b, 0:512] += res.results[2 * b]["pm"]
        out[b, 512:1024] += res.results[2 * b + 1]["pm"]
    return out


def kernel(**inputs):
    nc = _get_program()
    in_maps = _host_prep(inputs)
    res = run_bass_kernel_spmd(nc, in_maps, list(range(N_CORES)))
    return _assemble(res)


def kernel_traced(**inputs):
    """Like kernel() but returns (out, BassKernelResults) with HW timing."""
    nc = _get_program()
    in_maps = _host_prep(inputs)
    res = run_bass_kernel_spmd(nc, in_maps, list(range(N_CORES)), trace=True)
    return _assemble(res), res


# revision 19
# speedup vs baseline: 1.1024x; 1.0009x over previous
"""Trainium2 Bass kernel for nn_Block_75840532513234 (dense transformer block).

Sharding: 8 cores; core c handles batch b = c//2 and head-half hh = c%2
(8 of 16 heads). The two cores of a pair all-reduce the c_proj partial sums
(row-sharded c_proj) after each of the two sub-blocks, so both hold the full
residual stream; the pair's even core's output is used.

Layout strategy (per core):
  - x, x' kept natural (t-part, c-free); LN stats per-partition.
  - LN mean/scale folded into projection evictions as a rank-1 correction
    ((x@W - mu*colsum(W)) * rstd), so x is transposed raw on the PE.
  - LeakyAvg = banded matmul with two 128x128 per-head decay matrices
    (exp decay below ~1e-28 beyond 128 steps for beta >= 0.5).
  - Attention computed fully transposed: scoresT = KT.T-block @ KT,
    attT = exp(scoresT) (softmax needs no max subtraction: |scores| <= ~10),
    YT = [V|1].T @ attT gives Y and the denominators in one accumulation.
  - ln1/ln2 weights folded into W_k/W_v host-side; kscale via svec;
    out_scale/PM_COUNT folded into Pv host-side; vscale into v-normalize.
"""
import sys
import math

sys.path.insert(0, '/opt/trn_rl_repo')

import numpy as np

# ---------------------------------------------------------------------------
# Patches for this container's walrus build: it allows only ONE sync-wait per
# instruction, while Tile attaches several (final drain; ldweights). Split the
# extras onto standalone single-wait EventSemaphore instructions.
# ---------------------------------------------------------------------------
import concourse.tile as tile
import concourse.bass as bass
from concourse import mybir
from concourse.vector_clock import ScopedClock

_ev_ctr = [0]


def _split_multi_waits(nc):
    for f in nc.m.functions:
        for bb in f.blocks:
            il = bb.instructions
            i = 0
            while i < len(il):
                inst = il[i]
                si = inst.sync_info
                if si is not None and si.on_wait and len(si.on_wait) > 1:
                    waits = list(si.on_wait)
                    si.on_wait.clear()
                    si.on_wait.append(waits[-1])
                    for w in waits[:-1]:
                        _ev_ctr[0] += 1
                        ev = mybir.InstEventSemaphore(
                            name=f"EVSPLIT-{_ev_ctr[0]}", ins=[], outs=[])
                        ev.engine = inst.engine
                        ev.sync_info = mybir.SyncInfo(on_wait=[], on_update=[])
                        ev.sync_info.on_wait.append(w)
                        il.insert(i, ev)
                        i += 1
                i += 1


def _patched_drain_and_barrier(self, tick_clock, wait_clock):
    nc = self.nc
    drain_inst = nc.sync.drain()
    wait_clock.add_sem_waits(
        drain_inst.ins, ScopedClock({None: tick_clock.global_clock}))
    nc.all_engine_barrier()
    popped = nc._tile_sem_poison_stack.pop()
    assert popped is self._sem_poison
    nc.clear_and_free_semaphores(list(self.sems.allocated().values()))
    nc.all_engine_barrier()


tile.TileContext._drain_and_barrier = _patched_drain_and_barrier

_orig_tile_exit = tile.TileContext.__exit__


def _patched_tile_exit(self, *a, **k):
    r = _orig_tile_exit(self, *a, **k)
    _split_multi_waits(self.nc)
    return r


tile.TileContext.__exit__ = _patched_tile_exit

# NTFF profile hook (trimmed image lacks antenv.axon_hooks).
import types as _types

if "antenv.axon_hooks" not in sys.modules:
    _m = _types.ModuleType("antenv.axon_hooks")
    _hook_store = [None]

    def _set_hook(h):
        _hook_store[0] = h

    def _get_hook():
        if _hook_store[0] is None:
            try:
                if '/root/.axon_site' not in sys.path:
                    sys.path.insert(0, '/root/.axon_site')
                from trn_agent_boot.trn_boot import _ntff_profile_via_ctypes
                _hook_store[0] = _ntff_profile_via_ctypes(
                    '/opt/axon/libaxon_pjrt.so')
            except Exception:
                return None
        return _hook_store[0]

    _m.set_axon_ntff_profile_hook = _set_hook
    _m.get_axon_ntff_profile_hook = _get_hook
    sys.modules["antenv.axon_hooks"] = _m
    import antenv as _antenv
    _antenv.axon_hooks = _m

from concourse.bass_utils import run_bass_kernel_spmd  # noqa: E402

# ---------------------------------------------------------------------------
# Problem constants (hardcoded per the grading contract)
# ---------------------------------------------------------------------------
B, T, C, NH = 4, 1024, 1024, 16
HS = C // NH              # 64
NHL = 8                   # heads per core
CL = NHL * HS             # 512 local channels
EXP_SCALING = 10.0
KSCALE_MAX = float(np.log(2.0 ** 16 - 1))
N_CORES = 8
GROUPS = [[0, 1], [2, 3], [4, 5], [6, 7]]

f32 = mybir.dt.float32
f32r = mybir.dt.float32r
AF = mybir.ActivationFunctionType
ALU = mybir.AluOpType
AXL = mybir.AxisListType

NTB = T // 128            # 8 t-blocks
NCB = C // 128            # 8 c-blocks


def _build_program():
    nc = bass.Bass(num_devices=N_CORES)

    # ---- I/O ----
    x_in = nc.dram_tensor("x", [T, C], f32, kind="ExternalInput")
    wka_in = nc.dram_tensor("wka", [C, CL], f32, kind="ExternalInput")
    wva_in = nc.dram_tensor("wva", [C, CL], f32, kind="ExternalInput")
    wkm_in = nc.dram_tensor("wkm", [C, CL], f32, kind="ExternalInput")
    cpa_in = nc.dram_tensor("cpa", [CL, C], f32, kind="ExternalInput")
    cpm_in = nc.dram_tensor("cpm", [CL, C], f32, kind="ExternalInput")
    wkasb_in = nc.dram_tensor("wkasb", [128, CL], f32, kind="ExternalInput")
    wvasb_in = nc.dram_tensor("wvasb", [128, CL], f32, kind="ExternalInput")
    wkmsb_in = nc.dram_tensor("wkmsb", [128, CL], f32, kind="ExternalInput")
    dmata_in = nc.dram_tensor("dmata", [128, NHL * 256], f32, kind="ExternalInput")
    dmatm_in = nc.dram_tensor("dmatm", [128, NHL * 256], f32, kind="ExternalInput")
    sveca_in = nc.dram_tensor("sveca", [128, NHL], f32, kind="ExternalInput")
    svecm_in = nc.dram_tensor("svecm", [128, NHL], f32, kind="ExternalInput")
    coef1_in = nc.dram_tensor("coef1", [128, NHL], f32, kind="ExternalInput")
    vs_in = nc.dram_tensor("vs", [128, NHL], f32, kind="ExternalInput")
    pkt_in = nc.dram_tensor("pkt", [128, 4096], f32, kind="ExternalInput")
    pvo_in = nc.dram_tensor("pvo", [128, NHL * 8 * 66], f32, kind="ExternalInput")
    ident_in = nc.dram_tensor("ident", [128, 128], f32, kind="ExternalInput")
    maskt_in = nc.dram_tensor("maskt", [128, 4 * 512], f32, kind="ExternalInput")
    ones1_in = nc.dram_tensor("ones1", [1, 64], f32, kind="ExternalInput")
    vpad_in = nc.dram_tensor("vpad", [128, 16], f32, kind="ExternalInput")
    zcol_in = nc.dram_tensor("zcol", [128, 1], f32, kind="ExternalInput")
    epsv_in = nc.dram_tensor("epsv", [128, 1], f32, kind="ExternalInput")
    zrow_in = nc.dram_tensor("zrow", [1, 512], f32, kind="ExternalInput")

    xp_out = nc.dram_tensor("xp", [T, C], f32, kind="ExternalOutput")
    pm_out = nc.dram_tensor("pm", [512, C], f32, kind="ExternalOutput")

    cc1_in = nc.dram_tensor("cc1_in", [T, C], f32)
    cc1_out = nc.dram_tensor("cc1_out", [T, C], f32)
    cc2_in = nc.dram_tensor("cc2_in", [T, C], f32)
    cc2_out = nc.dram_tensor("cc2_out", [T, C], f32)

    with tile.TileContext(nc) as tc:
        # ---------------- persistent pools ----------------
        with tc.tile_pool(name="persist", bufs=1) as pp, \
             tc.tile_pool(name="work", bufs=1) as wp:
            # constants
            ident = pp.tile([128, 128], f32r, tag="ident")
            nc.sync.dma_start(ident[:], ident_in[:].bitcast(f32r))
            sveca = pp.tile([128, NHL], f32, tag="sveca")
            nc.sync.dma_start(sveca[:], sveca_in[:])
            svecm = pp.tile([128, NHL], f32, tag="svecm")
            nc.sync.dma_start(svecm[:], svecm_in[:])
            coef1 = pp.tile([128, NHL], f32, tag="coef1")
            nc.sync.dma_start(coef1[:], coef1_in[:])
            vsv = pp.tile([128, NHL], f32, tag="vsv")
            nc.sync.dma_start(vsv[:], vs_in[:])
            ones1 = pp.tile([1, 64], f32r, tag="ones1")
            nc.sync.dma_start(ones1[:], ones1_in[:].bitcast(f32r))
            vpad = pp.tile([128, 16], f32r, tag="vpad")
            nc.sync.dma_start(vpad[:], vpad_in[:].bitcast(f32r))
            zcol = pp.tile([128, 1], f32r, tag="zcol")
            nc.sync.dma_start(zcol[:], zcol_in[:].bitcast(f32r))
            epsv = pp.tile([128, 1], f32, tag="epsv")
            nc.sync.dma_start(epsv[:], epsv_in[:])
            maskt = pp.tile([128, 2048], f32, tag="maskt")
            nc.sync.dma_start(maskt[:], maskt_in[:])
            pvo = pp.tile([128, NHL * 8 * 66], f32r, tag="pvo")
            nc.sync.dma_start(pvo[:], pvo_in[:].bitcast(f32r))
            wsb = {}
            for nm, src in (("wka", wkasb_in), ("wva", wvasb_in),
                            ("wkm", wkmsb_in)):
                wsb[nm] = pp.tile([128, CL], f32, name=f"wsb_{nm}", tag=f"wsb_{nm}")
                nc.sync.dma_start(wsb[nm][:], src[:])

            # big persistent buffers (tags reused across the two branches)
            # X is f32r so PE can transpose it directly (rounds x once, ~1e-4)
            X = wp.tile([128, NTB * 1024], f32r, tag="X")        # x then out
            for tb in range(NTB):
                nc.sync.dma_start(
                    X[:, tb * 1024:(tb + 1) * 1024],
                    x_in[tb * 128:(tb + 1) * 128, :].bitcast(f32r))

            def branch(branch_id, wk_in, wsum_bc, dmat_in, svec, cc_in_t,
                       cc_out_t, wv_in=None, wvsum_bc=None, cp_in=None):
                """Emit one sub-block (context: with v; persistent: without).

                Returns nothing; accumulates residual into X in place.
                branch uses tiles tagged with shared tags so the second call
                reuses the first call's SBUF.
                """
                is_ctx = wv_in is not None

                # ---- LN stats ----
                negmu = []
                rstd = []
                with tc.tile_pool(name=f"st{branch_id}", bufs=2 * NTB) as sp, \
                     tc.tile_pool(name=f"sttmp{branch_id}", bufs=2) as stp:
                    for tb in range(NTB):
                        xs = X[:, tb * 1024:(tb + 1) * 1024].bitcast(f32)
                        s = stp.tile([128, 1], f32, tag="s")
                        nc.vector.reduce_sum(s[:], xs, axis=AXL.X)
                        sq = stp.tile([128, 1024], f32, tag="sq")
                        nc.vector.tensor_tensor(sq[:], xs, xs, ALU.mult)
                        ssq = stp.tile([128, 1], f32, tag="ssq")
                        nc.vector.reduce_sum(ssq[:], sq[:], axis=AXL.X)
                        nm = sp.tile([128, 1], f32, tag="negmu")
                        nc.vector.tensor_scalar_mul(nm[:], s[:], -1.0 / C)
                        mp = stp.tile([128, 1], f32, tag="mp")
                        nc.vector.tensor_scalar_mul(mp[:], s[:], 1.0 / C)
                        sn = stp.tile([128, 1], f32, tag="sn")
                        nc.vector.tensor_scalar_mul(sn[:], ssq[:], 1.0 / C)
                        var = stp.tile([128, 1], f32, tag="var")
                        nc.vector.scalar_tensor_tensor(
                            var[:], mp[:], nm[:], sn[:], ALU.mult, ALU.add)
                        lv = stp.tile([128, 1], f32, tag="lv")
                        nc.scalar.activation(lv[:], var[:], AF.Ln, bias=epsv[:])
                        rs = sp.tile([128, 1], f32, tag="rstd")
                        nc.scalar.activation(rs[:], lv[:], AF.Exp, scale=-0.5)
                        negmu.append(nm)
                        rstd.append(rs)

                    # ---- transpose + projections (grouped for PSUM) ----
                    kraw = wp.tile([128, NTB * CL], f32r, tag="kraw")
                    vraw = None
                    if is_ctx:
                        vraw = wp.tile([128, NTB * CL], f32, tag="vraw")
                    tgroups = [(0, 3), (3, 6), (6, 8)]
                    with tc.tile_pool(name=f"pj{branch_id}", bufs=1,
                                      space="PSUM") as pjp, \
                         tc.tile_pool(name=f"ptr{branch_id}", bufs=2,
                                      space="PSUM") as ptp, \
                         tc.tile_pool(name=f"w{branch_id}", bufs=4) as wpool, \
                         tc.tile_pool(name=f"xtc{branch_id}", bufs=2) as xtp, \
                         tc.tile_pool(name=f"ev{branch_id}", bufs=3) as evp:
                        for g0, g1 in tgroups:
                            gn = g1 - g0
                            psk = [pjp.tile([128, CL], f32,
                                            name=f"psk{g0}_{i}",
                                            tag=f"psk{i}")
                                   for i in range(gn)]
                            psv = [pjp.tile([128, CL], f32,
                                            name=f"psv{g0}_{i}",
                                            tag=f"psv{i}")
                                   for i in range(gn)] if is_ctx else None
                            for cb in range(NCB):
                                xtc = xtp.tile([128, gn * 128], f32r,
                                               tag="xtc")
                                for i, tb in enumerate(range(g0, g1)):
                                    ptr = ptp.tile([128, 128], f32, tag="ptr")
                                    nc.tensor.transpose(
                                        ptr[:].bitcast(f32r),
                                        X[:, tb * 1024 + cb * 128:
                                          tb * 1024 + cb * 128 + 128],
                                        ident[:])
                                    nc.scalar.copy(
                                        xtc[:, i * 128:(i + 1) * 128],
                                        ptr[:])
                                wk_c = wpool.tile([128, CL], f32r, tag="wk")
                                nc.sync.dma_start(
                                    wk_c[:],
                                    wk_in[cb * 128:(cb + 1) * 128, :]
                                    .bitcast(f32r))
                                wv_c = None
                                if is_ctx:
                                    wv_c = wpool.tile([128, CL], f32r,
                                                      tag="wv")
                                    nc.sync.dma_start(
                                        wv_c[:],
                                        wv_in[cb * 128:(cb + 1) * 128, :]
                                        .bitcast(f32r))
                                for i in range(gn):
                                    nc.tensor.matmul(
                                        psk[i][:],
                                        xtc[:, i * 128:(i + 1) * 128],
                                        wk_c[:], start=(cb == 0),
                                        stop=(cb == NCB - 1))
                                    if is_ctx:
                                        nc.tensor.matmul(
                                            psv[i][:],
                                            xtc[:, i * 128:(i + 1) * 128],
                                            wv_c[:], start=(cb == 0),
                                            stop=(cb == NCB - 1))
                            # evict with LN fold: (psum + negmu*wsum)*rstd
                            # kraw first: it gates the PE (LeakyAvg)
                            for i, tb in enumerate(range(g0, g1)):
                                tmp = evp.tile([128, CL], f32, tag="evt")
                                nc.vector.scalar_tensor_tensor(
                                    tmp[:], wsum_bc[:], negmu[tb][:],
                                    psk[i][:], ALU.mult, ALU.add)
                                nc.vector.tensor_scalar_mul(
                                    kraw[:, tb * CL:(tb + 1) * CL],
                                    tmp[:], rstd[tb][:])
                            for i, tb in enumerate(range(g0, g1)):
                                if is_ctx:
                                    tmp2 = evp.tile([128, CL], f32,
                                                    tag="evt2")
                                    nc.vector.scalar_tensor_tensor(
                                        tmp2[:], wvsum_bc[:], negmu[tb][:],
                                        psv[i][:], ALU.mult, ALU.add)
                                    nc.vector.tensor_scalar_mul(
                                        vraw[:, tb * CL:(tb + 1) * CL],
                                        tmp2[:], rstd[tb][:])

                # ---- v path (context only) ----
                vnorm = None
                if is_ctx:
                    vnorm = wp.tile([128, NTB * 528], f32r, tag="vnorm")
                    with tc.tile_pool(name="vtmp", bufs=3) as vtp, \
                         tc.tile_pool(name="vst", bufs=3) as vsp:
                        for tb in range(NTB):
                            vr = vraw[:, tb * CL:(tb + 1) * CL]
                            vsh = vtp.tile([128, CL], f32, tag="vsh")
                            nc.sync.dma_start(vsh[0:127, :],
                                              vraw[1:128, tb * CL:(tb + 1) * CL])
                            if tb < NTB - 1:
                                nc.sync.dma_start(
                                    vsh[127:128, :],
                                    vraw[0:1, (tb + 1) * CL:(tb + 2) * CL])
                            else:
                                nc.sync.dma_start(vsh[127:128, :], zrow_in[:])
                            # vmix = vraw + (1-coef)*(vsh - vraw), in place
                            nc.vector.tensor_tensor(vsh[:], vsh[:], vr,
                                                    ALU.subtract)
                            c_b = coef1[:].unsqueeze(2).broadcast_to(
                                (128, NHL, HS))
                            nc.vector.tensor_tensor(
                                vsh[:].rearrange("p (h d) -> p h d", h=NHL),
                                vsh[:].rearrange("p (h d) -> p h d", h=NHL),
                                c_b, ALU.mult)
                            nc.vector.tensor_tensor(vsh[:], vsh[:], vr,
                                                    ALU.add)
                            sq = vtp.tile([128, CL], f32, tag="vsq")
                            nc.vector.tensor_tensor(sq[:], vsh[:], vsh[:],
                                                    ALU.mult)
                            ssq = vsp.tile([128, NHL], f32, tag="vssq")
                            nc.vector.reduce_sum(
                                ssq[:], sq[:].rearrange("p (h d) -> p h d",
                                                        h=NHL), axis=AXL.X)
                            lnv = vsp.tile([128, NHL], f32, tag="vlnv")
                            nc.scalar.activation(lnv[:], ssq[:], AF.Ln)
                            rn = vsp.tile([128, NHL], f32, tag="vrn")
                            nc.scalar.activation(rn[:], lnv[:], AF.Exp,
                                                 scale=-0.5)
                            rns = vsp.tile([128, NHL], f32, tag="vrns")
                            nc.vector.tensor_tensor(rns[:], rn[:], vsv[:],
                                                    ALU.mult)
                            rns_b = rns[:].unsqueeze(2).broadcast_to(
                                (128, NHL, HS))
                            vslice = vnorm[:, tb * 528:(tb + 1) * 528]
                            nc.vector.tensor_tensor(
                                vslice.rearrange("p (h c) -> p h c",
                                                 h=NHL)[:, :, 0:64],
                                vsh[:].rearrange("p (h d) -> p h d", h=NHL),
                                rns_b, ALU.mult)
                            nc.sync.dma_start(
                                vslice.rearrange("p (h c) -> p h c",
                                                 h=NHL)[:, :, 64:66],
                                vpad[:].rearrange("p (h t) -> p h t", h=NHL))

                # ---- LeakyAvg + normalize + transpose -> ktall ----
                dmat = wp.tile([128, NHL * 256], f32r, tag="dmat")
                nc.sync.dma_start(dmat[:], dmat_in[:].bitcast(f32r))
                ktall = wp.tile([128, 4096], f32r, tag="ktall")
                with tc.tile_pool(name=f"lv{branch_id}", bufs=2,
                                  space="PSUM") as lvp, \
                     tc.tile_pool(name=f"ltr{branch_id}", bufs=2,
                                  space="PSUM") as ltp, \
                     tc.tile_pool(name=f"le{branch_id}", bufs=3) as lep, \
                     tc.tile_pool(name=f"ls{branch_id}", bufs=4) as lsp:
                    for h in range(NHL):
                        pl = lvp.tile([128, CL], f32, tag="pl")
                        kview = kraw[:].rearrange("p (b r) -> p b r", r=CL)
                        rhs_all = kview[:, :, h * 64:h * 64 + 64]
                        nc.tensor.matmul(
                            pl[:], dmat[:, h * 256:h * 256 + 128],
                            rhs_all, start=True, stop=False)
                        rhs_prev = kview[:, 0:7, h * 64:h * 64 + 64]
                        nc.tensor.matmul(
                            pl[:, 64:512], dmat[:, h * 256 + 128:h * 256 + 256],
                            rhs_prev, start=False, stop=True)
                        lraw = lep.tile([128, CL], f32, tag="lraw")
                        nc.scalar.copy(lraw[:], pl[:])
                        sq = lep.tile([128, CL], f32, tag="lsq")
                        nc.vector.tensor_tensor(sq[:], lraw[:], lraw[:],
                                                ALU.mult)
                        ssq = lsp.tile([128, 8], f32, tag="lssq")
                        nc.vector.reduce_sum(
                            ssq[:], sq[:].rearrange("p (b d) -> p b d", d=64),
                            axis=AXL.X)
                        lnv = lsp.tile([128, 8], f32, tag="llnv")
                        nc.scalar.activation(lnv[:], ssq[:], AF.Ln)
                        rn = lsp.tile([128, 8], f32, tag="lrn")
                        nc.scalar.activation(rn[:], lnv[:], AF.Exp, scale=-0.5)
                        rns = lsp.tile([128, 8], f32, tag="lrns")
                        nc.vector.tensor_scalar_mul(rns[:], rn[:],
                                                    svec[:, h:h + 1])
                        kfeat = lep.tile([128, CL], f32r, tag="kfeat")
                        rb = rns[:].unsqueeze(2).broadcast_to((128, 8, 64))
                        nc.vector.tensor_tensor(
                            kfeat[:].rearrange("p (b d) -> p b d", d=64),
                            lraw[:].rearrange("p (b d) -> p b d", d=64),
                            rb, ALU.mult)
                        # transpose 8 blocks of (128,64) -> (64,128)
                        pbase = (h % 2) * 64
                        fbase = (h // 2) * 1024
                        for half in range(2):
                            ptr = ltp.tile([64, 512], f32, tag="ktr")
                            for q in range(4):
                                blk = half * 4 + q
                                nc.tensor.transpose(
                                    ptr[:, q * 128:(q + 1) * 128]
                                    .bitcast(f32r),
                                    kfeat[:, blk * 64:(blk + 1) * 64],
                                    ident[:])
                            nc.scalar.copy(
                                ktall[pbase:pbase + 64,
                                      fbase + half * 512:fbase + half * 512
                                      + 512],
                                ptr[:])

                # ---- attention ----
                ytall = wp.tile([128, 4096], f32r, tag="ytall")
                with tc.tile_pool(name=f"as{branch_id}", bufs=4,
                                  space="PSUM") as asp, \
                     tc.tile_pool(name=f"ay{branch_id}", bufs=3,
                                  space="PSUM") as ayp, \
                     tc.tile_pool(name=f"ab{branch_id}", bufs=1,
                                  space="PSUM") as abp, \
                     tc.tile_pool(name=f"at{branch_id}", bufs=5) as atp, \
                     tc.tile_pool(name=f"ar{branch_id}", bufs=2) as arp:
                    for h in range(NHL):
                        pbase = (h % 2) * 64
                        fbase = (h // 2) * 1024
                        kt_h = ktall[pbase:pbase + 64, fbase:fbase + 1024]
                        for qc in range(2):
                            py = ayp.tile([66, 512], f32, tag="py")
                            njb = 4 if (is_ctx and qc == 0) else 8
                            for jb in range(njb):
                                ps = asp.tile([128, 512], f32, tag="ps")
                                if is_ctx:
                                    lhs_sc = kt_h[:, jb * 128:(jb + 1) * 128]
                                else:
                                    lhs_sc = pktall[pbase:pbase + 64,
                                                    fbase + jb * 128:
                                                    fbase + (jb + 1) * 128]
                                nc.tensor.matmul(
                                    ps[:], lhs_sc,
                                    kt_h[:, qc * 512:(qc + 1) * 512],
                                    start=True, stop=True)
                                att = atp.tile([128, 512], f32r, tag="att")
                                r = jb - qc * 4
                                if is_ctx and r >= 0:
                                    araw = atp.tile([128, 512], f32,
                                                    tag="araw")
                                    nc.scalar.activation(araw[:], ps[:],
                                                         AF.Exp)
                                    nc.vector.tensor_tensor(
                                        att[:], araw[:],
                                        maskt[:, r * 512:(r + 1) * 512],
                                        ALU.mult)
                                else:
                                    nc.scalar.activation(att[:], ps[:],
                                                         AF.Exp)
                                if is_ctx:
                                    lhs_v = vnorm[:, jb * 528 + h * 66:
                                                  jb * 528 + (h + 1) * 66]
                                else:
                                    lhs_v = pvo[:, h * 528 + jb * 66:
                                                h * 528 + (jb + 1) * 66]
                                nc.tensor.matmul(py[:], lhs_v, att[:],
                                                 start=(jb == 0),
                                                 stop=(jb == njb - 1))
                            # 1/denom = exp(-ln(denom)) on ScalarE (2 ULP;
                            # stays in the natural_log_exp table set)
                            lrow = arp.tile([1, 512], f32, tag="lrow")
                            nc.scalar.activation(lrow[:], py[64:65, :], AF.Ln)
                            rrow = arp.tile([1, 512], f32r, tag="rrow")
                            nc.scalar.activation(rrow[:], lrow[:], AF.Exp,
                                                 scale=-1.0)
                            pb = abp.tile([64, 512], f32, tag="pb")
                            nc.tensor.matmul(pb[:], ones1[:], rrow[:],
                                             start=True, stop=True)
                            bcs = atp.tile([64, 512], f32, tag="bcs")
                            nc.scalar.copy(bcs[:], pb[:])
                            nc.vector.tensor_tensor(
                                ytall[pbase:pbase + 64,
                                      fbase + qc * 512:fbase + qc * 512 + 512],
                                py[0:64, :], bcs[:], ALU.mult)
                        if is_ctx:
                            # zero out the t=0 column (query 0 has no keys)
                            nc.sync.dma_start(
                                ytall[pbase:pbase + 64, fbase:fbase + 1],
                                zcol[0:64, :])

                # ---- c_proj -> collective -> residual into X ----
                # context: chunked AllReduce (both cores need full x').
                # persistent: ReduceScatter; each core finishes only its
                # half of the output rows (host stitches halves).
                with tc.tile_pool(name=f"cp{branch_id}", bufs=2,
                                  space="PSUM") as cpp, \
                     tc.tile_pool(name=f"cw{branch_id}", bufs=2) as cwp, \
                     tc.tile_pool(name=f"cs{branch_id}", bufs=2) as csp:
                    cpw = [None] * 4
                    for cb in range(4):
                        cpw[cb] = cwp.tile([128, 1024], f32r, name=f"cpw{cb}", tag=f"cpw{cb}")
                        nc.sync.dma_start(
                            cpw[cb][:],
                            cp_in[cb * 128:(cb + 1) * 128, :].bitcast(f32r))
                    for tb in range(NTB):
                        stage = csp.tile([128, 1024], f32, tag="cstage")
                        for co in range(2):
                            pc = cpp.tile([128, 512], f32, tag="pc")
                            for cb in range(4):
                                nc.tensor.matmul(
                                    pc[:],
                                    ytall[:, cb * 1024 + tb * 128:
                                          cb * 1024 + tb * 128 + 128],
                                    cpw[cb][:, co * 512:(co + 1) * 512],
                                    start=(cb == 0), stop=(cb == 3))
                            nc.scalar.copy(stage[:, co * 512:(co + 1) * 512],
                                           pc[:])
                        nc.sync.dma_start(
                            cc_in_t[tb * 128:(tb + 1) * 128, :], stage[:])
                        if is_ctx and tb in (3, NTB - 1):
                            # half-chunk AllReduce as soon as rows are staged
                            lo = 0 if tb == 3 else 512
                            nc.gpsimd.collective_compute(
                                "AllReduce", ALU.add, replica_groups=GROUPS,
                                ins=[cc_in_t[lo:lo + 512, :]],
                                outs=[cc_out_t[lo:lo + 512, :]])
                    if is_ctx:
                        for tb in range(NTB):
                            back = csp.tile([128, 1024], f32, tag="cback")
                            nc.sync.dma_start(
                                back[:], cc_out_t[tb * 128:(tb + 1) * 128, :])
                            nc.vector.tensor_tensor(
                                X[:, tb * 1024:(tb + 1) * 1024],
                                X[:, tb * 1024:(tb + 1) * 1024].bitcast(f32),
                                back[:], ALU.add)
                    else:
                        # each core gets its pair-rank's half of the summed
                        # pm; host adds it onto the matching half of x'.
                        nc.gpsimd.collective_compute(
                            "ReduceScatter", ALU.add, replica_groups=GROUPS,
                            ins=[cc_in_t[:]], outs=[cc_out_t[0:512, :]])
                        for i in range(4):
                            back = csp.tile([128, 1024], f32, tag="cback")
                            nc.sync.dma_start(
                                back[:], cc_out_t[i * 128:(i + 1) * 128, :])
                            nc.sync.dma_start(
                                pm_out[i * 128:(i + 1) * 128, :], back[:])

            # -------- context branch --------
            pktall = None
            branch(0, wka_in, wsb["wka"], dmata_in, sveca, cc1_in, cc1_out,
                   wv_in=wva_in, wvsum_bc=wsb["wva"], cp_in=cpa_in)

            # x' is final except for the pm half-add done on host
            nc.sync.dma_start(
                xp_out[:].bitcast(f32r).rearrange("(b p) c -> p b c", p=128),
                X[:].rearrange("p (b c) -> p b c", b=NTB))

            # load persistent-memory keys into the vnorm slot (context-only)
            pktall = wp.tile([128, 4224], f32r, tag="vnorm")
            nc.sync.dma_start(pktall[:, 0:4096], pkt_in[:].bitcast(f32r))

            # -------- persistent branch --------
            branch(1, wkm_in, wsb["wkm"], dmatm_in, svecm, cc2_in, cc2_out,
                   cp_in=cpm_in)



    return nc


_prog_cache = {}


def _get_program():
    if "nc" not in _prog_cache:
        _prog_cache["nc"] = _build_program()
    return _prog_cache["nc"]


def _host_prep(inputs):
    """Build the 8 per-core input maps from the full-problem inputs."""
    x = np.asarray(inputs["x"], np.float32)
    ln1 = np.asarray(inputs["ln1_w"], np.float32)
    ln2 = np.asarray(inputs["ln2_w"], np.float32)
    Wk_a = np.asarray(inputs["Wk_a"], np.float32)
    Wv_a = np.asarray(inputs["Wv_a"], np.float32)
    cproj_a = np.asarray(inputs["cproj_a"], np.float32)
    beta_a = np.asarray(inputs["beta_a"], np.float32).reshape(NH)
    kscale_a = np.asarray(inputs["kscale_a"], np.float32).reshape(NH)
    vcoef = np.asarray(inputs["vcoef"], np.float32).reshape(NH)
    vscale = np.asarray(inputs["vscale"], np.float32).reshape(NH)
    Wk_m = np.asarray(inputs["Wk_m"], np.float32)
    beta_m = np.asarray(inputs["beta_m"], np.float32).reshape(NH)
    kscale_m = np.asarray(inputs["kscale_m"], np.float32).reshape(NH)
    Pk = np.asarray(inputs["Pk"], np.float32)
    Pv = np.asarray(inputs["Pv"], np.float32)
    out_scale = np.asarray(inputs["out_scale"], np.float32).reshape(NH)
    cproj_m = np.asarray(inputs["cproj_m"], np.float32)

    J, I = np.meshgrid(np.arange(128), np.arange(128), indexing="ij")

    def dmats(beta, heads):
        out = np.zeros((128, NHL * 256), np.float32)
        for i, h in enumerate(heads):
            b = abs(float(beta[h])) * EXP_SCALING
            out[:, i * 256:i * 256 + 128] = np.where(
                I >= J, np.exp(-(I - J) * b), 0.0)
            out[:, i * 256 + 128:i * 256 + 256] = np.exp(-((I + 128) - J) * b)
        return out

    # context diagonal masks: mask_r[jl, ql] = 1 if jl + r*128 < ql
    maskt = np.zeros((128, 2048), np.float32)
    jl = np.arange(128)[:, None]
    ql = np.arange(512)[None, :]
    for r in range(4):
        maskt[:, r * 512:(r + 1) * 512] = (jl + r * 128 < ql)

    vpad = np.zeros((128, 16), np.float32)
    vpad[:, 0::2] = 1.0

    base = {
        "ident": np.eye(128, dtype=np.float32),
        "maskt": maskt,
        "ones1": np.ones((1, 64), np.float32),
        "vpad": vpad,
        "zcol": np.zeros((128, 1), np.float32),
        "epsv": np.full((128, 1), 1e-5, np.float32),
        "zrow": np.zeros((1, 512), np.float32),
    }

    in_maps = []
    for c in range(N_CORES):
        b = c // 2
        hh = c % 2
        cols = slice(hh * CL, (hh + 1) * CL)
        heads = list(range(hh * NHL, hh * NHL + NHL))

        wka = (Wk_a * ln1[None, :])[cols].T.copy()      # (C, 512)
        wva = (Wv_a * ln1[None, :])[cols].T.copy()
        wkm = (Wk_m * ln2[None, :])[cols].T.copy()

        sva = np.exp(np.minimum(1.0 * EXP_SCALING * kscale_a[heads],
                                KSCALE_MAX))
        svm = np.exp(np.minimum(2.0 * EXP_SCALING * kscale_m[heads],
                                KSCALE_MAX))
        vs = np.exp(EXP_SCALING * vscale[heads])
        c1 = 1.0 - vcoef[heads]
        osc = np.exp(EXP_SCALING * out_scale[heads]) / Pk.shape[0]

        pkt = np.zeros((128, 4096), np.float32)
        pvo = np.zeros((128, NHL * 8 * 66), np.float32)
        for i, h in enumerate(heads):
            pb_ = (i % 2) * 64
            fb = (i // 2) * 1024
            pkt[pb_:pb_ + 64, fb:fb + 1024] = Pk[0, 0, h].T
            for pb2 in range(8):
                col = i * 528 + pb2 * 66
                pvo[:, col:col + 64] = Pv[0, 0, h, pb2 * 128:(pb2 + 1) * 128,
                                          :] * osc[i]
                pvo[:, col + 64] = 1.0
                pvo[:, col + 65] = 0.0

        m = dict(base)
        m.update({
            "x": np.ascontiguousarray(x[b]),
            "wka": np.ascontiguousarray(wka),
            "wva": np.ascontiguousarray(wva),
            "wkm": np.ascontiguousarray(wkm),
            "cpa": np.ascontiguousarray(cproj_a[:, cols].T),
            "cpm": np.ascontiguousarray(cproj_m[:, cols].T),
            "wkasb": np.broadcast_to(wka.sum(0), (128, CL)).copy(),
            "wvasb": np.broadcast_to(wva.sum(0), (128, CL)).copy(),
            "wkmsb": np.broadcast_to(wkm.sum(0), (128, CL)).copy(),
            "dmata": dmats(beta_a, heads),
            "dmatm": dmats(beta_m, heads),
            "sveca": np.broadcast_to(sva, (128, NHL)).copy(),
            "svecm": np.broadcast_to(svm, (128, NHL)).copy(),
            "coef1": np.broadcast_to(c1, (128, NHL)).copy(),
            "vs": np.broadcast_to(vs, (128, NHL)).copy(),
            "pkt": pkt,
            "pvo": pvo,
        })
        in_maps.append(m)
    return in_maps


def _assemble(res):
    out = np.empty((B, T, C), np.float32)
    for b in range(B):
        xp = res.results[2 * b]["xp"]
        out[b] = xp
        out[b, 0:512] += res.results[2 * b]["pm"]
        out[b, 512:1024] += res.results[2 * b + 1]["pm"]
    return out


def kernel(**inputs):
    nc = _get_program()
    in_maps = _host_prep(inputs)
    res = run_bass_kernel_spmd(nc, in_maps, list(range(N_CORES)))
    return _assemble(res)


def kernel_traced(**inputs):
    """Like kernel() but returns (out, BassKernelResults) with HW timing."""
    nc = _get_program()
    in_maps = _host_prep(inputs)
    res = run_bass_kernel_spmd(nc, in_maps, list(range(N_CORES)), trace=True)
    return _assemble(res), res


# revision 22
# speedup vs baseline: 1.2036x; 1.0918x over previous
"""Trainium2 Bass kernel for nn_Block_75840532513234 (dense transformer block).

Sharding: 8 cores; core c handles batch b = c//2 and head-half hh = c%2
(8 of 16 heads). The two cores of a pair all-reduce the c_proj partial sums
(row-sharded c_proj) after each of the two sub-blocks, so both hold the full
residual stream; the pair's even core's output is used.

Layout strategy (per core):
  - x, x' kept natural (t-part, c-free); LN stats per-partition.
  - LN mean/scale folded into projection evictions as a rank-1 correction
    ((x@W - mu*colsum(W)) * rstd), so x is transposed raw on the PE.
  - LeakyAvg = banded matmul with two 128x128 per-head decay matrices
    (exp decay below ~1e-28 beyond 128 steps for beta >= 0.5).
  - Attention computed fully transposed: scoresT = KT.T-block @ KT,
    attT = exp(scoresT) (softmax needs no max subtraction: |scores| <= ~10),
    YT = [V|1].T @ attT gives Y and the denominators in one accumulation.
  - ln1/ln2 weights folded into W_k/W_v host-side; kscale via svec;
    out_scale/PM_COUNT folded into Pv host-side; vscale into v-normalize.
"""
import sys
import math

sys.path.insert(0, '/opt/trn_rl_repo')

import numpy as np

# ---------------------------------------------------------------------------
# Patches for this container's walrus build: it allows only ONE sync-wait per
# instruction, while Tile attaches several (final drain; ldweights). Split the
# extras onto standalone single-wait EventSemaphore instructions.
# ---------------------------------------------------------------------------
import concourse.tile as tile
import concourse.bass as bass
from concourse import mybir
from concourse.vector_clock import ScopedClock

_ev_ctr = [0]


def _split_multi_waits(nc):
    for f in nc.m.functions:
        for bb in f.blocks:
            il = bb.instructions
            i = 0
            while i < len(il):
                inst = il[i]
                si = inst.sync_info
                if si is not None and si.on_wait and len(si.on_wait) > 1:
                    waits = list(si.on_wait)
                    si.on_wait.clear()
                    si.on_wait.append(waits[-1])
                    for w in waits[:-1]:
                        _ev_ctr[0] += 1
                        ev = mybir.InstEventSemaphore(
                            name=f"EVSPLIT-{_ev_ctr[0]}", ins=[], outs=[])
                        ev.engine = inst.engine
                        ev.sync_info = mybir.SyncInfo(on_wait=[], on_update=[])
                        ev.sync_info.on_wait.append(w)
                        il.insert(i, ev)
                        i += 1
                i += 1


def _patched_drain_and_barrier(self, tick_clock, wait_clock):
    nc = self.nc
    drain_inst = nc.sync.drain()
    wait_clock.add_sem_waits(
        drain_inst.ins, ScopedClock({None: tick_clock.global_clock}))
    nc.all_engine_barrier()
    popped = nc._tile_sem_poison_stack.pop()
    assert popped is self._sem_poison
    nc.clear_and_free_semaphores(list(self.sems.allocated().values()))
    nc.all_engine_barrier()


tile.TileContext._drain_and_barrier = _patched_drain_and_barrier

_orig_tile_exit = tile.TileContext.__exit__


def _patched_tile_exit(self, *a, **k):
    r = _orig_tile_exit(self, *a, **k)
    _split_multi_waits(self.nc)
    return r


tile.TileContext.__exit__ = _patched_tile_exit

# NTFF profile hook (trimmed image lacks antenv.axon_hooks).
import types as _types

if "antenv.axon_hooks" not in sys.modules:
    _m = _types.ModuleType("antenv.axon_hooks")
    _hook_store = [None]

    def _set_hook(h):
        _hook_store[0] = h

    def _get_hook():
        if _hook_store[0] is None:
            try:
                if '/root/.axon_site' not in sys.path:
                    sys.path.insert(0, '/root/.axon_site')
                from trn_agent_boot.trn_boot import _ntff_profile_via_ctypes
                _hook_store[0] = _ntff_profile_via_ctypes(
                    '/opt/axon/libaxon_pjrt.so')
            except Exception:
                return None
        return _hook_store[0]

    _m.set_axon_ntff_profile_hook = _set_hook
    _m.get_axon_ntff_profile_hook = _get_hook
    sys.modules["antenv.axon_hooks"] = _m
    import antenv as _antenv
    _antenv.axon_hooks = _m

from concourse.bass_utils import run_bass_kernel_spmd  # noqa: E402

# ---------------------------------------------------------------------------
# Problem constants (hardcoded per the grading contract)
# ---------------------------------------------------------------------------
B, T, C, NH = 4, 1024, 1024, 16
HS = C // NH              # 64
NHL = 8                   # heads per core
CL = NHL * HS             # 512 local channels
EXP_SCALING = 10.0
KSCALE_MAX = float(np.log(2.0 ** 16 - 1))
N_CORES = 8
GROUPS = [[0, 1], [2, 3], [4, 5], [6, 7]]

f32 = mybir.dt.float32
f32r = mybir.dt.float32r
AF = mybir.ActivationFunctionType
ALU = mybir.AluOpType
AXL = mybir.AxisListType

NTB = T // 128            # 8 t-blocks
NCB = C // 128            # 8 c-blocks


def _build_program():
    nc = bass.Bass(num_devices=N_CORES)

    # ---- I/O ----
    x_in = nc.dram_tensor("x", [T, C], f32, kind="ExternalInput")
    wka_in = nc.dram_tensor("wka", [C, CL], f32, kind="ExternalInput")
    wva_in = nc.dram_tensor("wva", [C, CL], f32, kind="ExternalInput")
    wkm_in = nc.dram_tensor("wkm", [C, CL], f32, kind="ExternalInput")
    cpa_in = nc.dram_tensor("cpa", [CL, C], f32, kind="ExternalInput")
    cpm_in = nc.dram_tensor("cpm", [CL, C], f32, kind="ExternalInput")
    wkasb_in = nc.dram_tensor("wkasb", [128, CL], f32, kind="ExternalInput")
    wvasb_in = nc.dram_tensor("wvasb", [128, CL], f32, kind="ExternalInput")
    wkmsb_in = nc.dram_tensor("wkmsb", [128, CL], f32, kind="ExternalInput")
    dmata_in = nc.dram_tensor("dmata", [128, NHL * 256], f32, kind="ExternalInput")
    dmatm_in = nc.dram_tensor("dmatm", [128, NHL * 256], f32, kind="ExternalInput")
    sveca_in = nc.dram_tensor("sveca", [128, NHL], f32, kind="ExternalInput")
    svecm_in = nc.dram_tensor("svecm", [128, NHL], f32, kind="ExternalInput")
    coef1_in = nc.dram_tensor("coef1", [128, NHL], f32, kind="ExternalInput")
    vs_in = nc.dram_tensor("vs", [128, NHL], f32, kind="ExternalInput")
    pkt_in = nc.dram_tensor("pkt", [128, 4096], f32, kind="ExternalInput")
    pvo_in = nc.dram_tensor("pvo", [128, NHL * 8 * 66], f32, kind="ExternalInput")
    ident_in = nc.dram_tensor("ident", [128, 128], f32, kind="ExternalInput")
    maskt_in = nc.dram_tensor("maskt", [128, 4 * 512], f32, kind="ExternalInput")
    ones1_in = nc.dram_tensor("ones1", [1, 64], f32, kind="ExternalInput")
    vpad_in = nc.dram_tensor("vpad", [128, 16], f32, kind="ExternalInput")
    zcol_in = nc.dram_tensor("zcol", [128, 1], f32, kind="ExternalInput")
    epsv_in = nc.dram_tensor("epsv", [128, 1], f32, kind="ExternalInput")
    zrow_in = nc.dram_tensor("zrow", [1, 512], f32, kind="ExternalInput")
    smat_in = nc.dram_tensor("smat", [128, 256], f32, kind="ExternalInput")

    xp_out = nc.dram_tensor("xp", [T, C], f32, kind="ExternalOutput")
    pm_out = nc.dram_tensor("pm", [512, C], f32, kind="ExternalOutput")

    cc1_in = nc.dram_tensor("cc1_in", [T, C], f32)
    cc1_out = nc.dram_tensor("cc1_out", [T, C], f32)
    cc2_in = nc.dram_tensor("cc2_in", [T, C], f32)
    cc2_out = nc.dram_tensor("cc2_out", [T, C], f32)

    with tile.TileContext(nc) as tc:
        # ---------------- persistent pools ----------------
        with tc.tile_pool(name="persist", bufs=1) as pp, \
             tc.tile_pool(name="work", bufs=1) as wp:
            # constants
            ident = pp.tile([128, 128], f32r, tag="ident")
            nc.sync.dma_start(ident[:], ident_in[:].bitcast(f32r))
            sveca = pp.tile([128, NHL], f32, tag="sveca")
            nc.sync.dma_start(sveca[:], sveca_in[:])
            svecm = pp.tile([128, NHL], f32, tag="svecm")
            nc.sync.dma_start(svecm[:], svecm_in[:])
            coef1 = pp.tile([128, NHL], f32, tag="coef1")
            nc.sync.dma_start(coef1[:], coef1_in[:])
            vsv = pp.tile([128, NHL], f32, tag="vsv")
            nc.sync.dma_start(vsv[:], vs_in[:])
            ones1 = pp.tile([1, 64], f32r, tag="ones1")
            nc.sync.dma_start(ones1[:], ones1_in[:].bitcast(f32r))
            vpad = pp.tile([128, 16], f32r, tag="vpad")
            nc.sync.dma_start(vpad[:], vpad_in[:].bitcast(f32r))
            zcol = pp.tile([128, 1], f32r, tag="zcol")
            nc.sync.dma_start(zcol[:], zcol_in[:].bitcast(f32r))
            epsv = pp.tile([128, 1], f32, tag="epsv")
            nc.sync.dma_start(epsv[:], epsv_in[:])
            smat = pp.tile([128, 256], f32r, tag="smat")
            nc.sync.dma_start(smat[:], smat_in[:].bitcast(f32r))
            maskt = pp.tile([128, 2048], f32, tag="maskt")
            nc.sync.dma_start(maskt[:], maskt_in[:])
            pvo = pp.tile([128, NHL * 8 * 66], f32r, tag="pvo")
            nc.sync.dma_start(pvo[:], pvo_in[:].bitcast(f32r))
            wsb = {}
            for nm, src in (("wka", wkasb_in), ("wva", wvasb_in),
                            ("wkm", wkmsb_in)):
                wsb[nm] = pp.tile([128, CL], f32, name=f"wsb_{nm}", tag=f"wsb_{nm}")
                nc.sync.dma_start(wsb[nm][:], src[:])

            # big persistent buffers (tags reused across the two branches)
            # X is f32r so PE can transpose it directly (rounds x once, ~1e-4)
            X = wp.tile([128, NTB * 1024], f32r, tag="X")        # x then out
            for tb in range(NTB):
                nc.sync.dma_start(
                    X[:, tb * 1024:(tb + 1) * 1024],
                    x_in[tb * 128:(tb + 1) * 128, :].bitcast(f32r))

            def branch(branch_id, wk_in, wsum_bc, dmat_in, svec, cc_in_t,
                       cc_out_t, wv_in=None, wvsum_bc=None, cp_in=None):
                """Emit one sub-block (context: with v; persistent: without).

                Returns nothing; accumulates residual into X in place.
                branch uses tiles tagged with shared tags so the second call
                reuses the first call's SBUF.
                """
                is_ctx = wv_in is not None

                # ---- LN stats ----
                negmu = []
                rstd = []
                with tc.tile_pool(name=f"st{branch_id}", bufs=2 * NTB) as sp, \
                     tc.tile_pool(name=f"sttmp{branch_id}", bufs=2) as stp:
                    for tb in range(NTB):
                        xs = X[:, tb * 1024:(tb + 1) * 1024].bitcast(f32)
                        s = stp.tile([128, 1], f32, tag="s")
                        nc.vector.reduce_sum(s[:], xs, axis=AXL.X)
                        sq = stp.tile([128, 1024], f32, tag="sq")
                        nc.vector.tensor_tensor(sq[:], xs, xs, ALU.mult)
                        ssq = stp.tile([128, 1], f32, tag="ssq")
                        nc.vector.reduce_sum(ssq[:], sq[:], axis=AXL.X)
                        nm = sp.tile([128, 1], f32, tag="negmu")
                        nc.vector.tensor_scalar_mul(nm[:], s[:], -1.0 / C)
                        mp = stp.tile([128, 1], f32, tag="mp")
                        nc.vector.tensor_scalar_mul(mp[:], s[:], 1.0 / C)
                        sn = stp.tile([128, 1], f32, tag="sn")
                        nc.vector.tensor_scalar_mul(sn[:], ssq[:], 1.0 / C)
                        var = stp.tile([128, 1], f32, tag="var")
                        nc.vector.scalar_tensor_tensor(
                            var[:], mp[:], nm[:], sn[:], ALU.mult, ALU.add)
                        lv = stp.tile([128, 1], f32, tag="lv")
                        nc.scalar.activation(lv[:], var[:], AF.Ln, bias=epsv[:])
                        rs = sp.tile([128, 1], f32, tag="rstd")
                        nc.scalar.activation(rs[:], lv[:], AF.Exp, scale=-0.5)
                        negmu.append(nm)
                        rstd.append(rs)

                    # ---- transpose + projections (grouped for PSUM) ----
                    kraw = wp.tile([128, NTB * CL], f32r, tag="kraw")
                    vraw = None
                    if is_ctx:
                        vraw = wp.tile([128, NTB * CL], f32r, tag="vraw")
                    tgroups = [(0, 3), (3, 6), (6, 8)]
                    with tc.tile_pool(name=f"pj{branch_id}", bufs=1,
                                      space="PSUM") as pjp, \
                         tc.tile_pool(name=f"ptr{branch_id}", bufs=2,
                                      space="PSUM") as ptp, \
                         tc.tile_pool(name=f"w{branch_id}", bufs=4) as wpool, \
                         tc.tile_pool(name=f"xtc{branch_id}", bufs=2) as xtp, \
                         tc.tile_pool(name=f"ev{branch_id}", bufs=3) as evp:
                        for g0, g1 in tgroups:
                            gn = g1 - g0
                            psk = [pjp.tile([128, CL], f32,
                                            name=f"psk{g0}_{i}",
                                            tag=f"psk{i}")
                                   for i in range(gn)]
                            psv = [pjp.tile([128, CL], f32,
                                            name=f"psv{g0}_{i}",
                                            tag=f"psv{i}")
                                   for i in range(gn)] if is_ctx else None
                            for cb in range(NCB):
                                xtc = xtp.tile([128, gn * 128], f32r,
                                               tag="xtc")
                                for i, tb in enumerate(range(g0, g1)):
                                    ptr = ptp.tile([128, 128], f32, tag="ptr")
                                    nc.tensor.transpose(
                                        ptr[:].bitcast(f32r),
                                        X[:, tb * 1024 + cb * 128:
                                          tb * 1024 + cb * 128 + 128],
                                        ident[:])
                                    nc.scalar.copy(
                                        xtc[:, i * 128:(i + 1) * 128],
                                        ptr[:])
                                wk_c = wpool.tile([128, CL], f32r, tag="wk")
                                nc.sync.dma_start(
                                    wk_c[:],
                                    wk_in[cb * 128:(cb + 1) * 128, :]
                                    .bitcast(f32r))
                                wv_c = None
                                if is_ctx:
                                    wv_c = wpool.tile([128, CL], f32r,
                                                      tag="wv")
                                    nc.sync.dma_start(
                                        wv_c[:],
                                        wv_in[cb * 128:(cb + 1) * 128, :]
                                        .bitcast(f32r))
                                for i in range(gn):
                                    nc.tensor.matmul(
                                        psk[i][:],
                                        xtc[:, i * 128:(i + 1) * 128],
                                        wk_c[:], start=(cb == 0),
                                        stop=(cb == NCB - 1))
                                    if is_ctx:
                                        nc.tensor.matmul(
                                            psv[i][:],
                                            xtc[:, i * 128:(i + 1) * 128],
                                            wv_c[:], start=(cb == 0),
                                            stop=(cb == NCB - 1))
                            # evict with LN fold: (psum + negmu*wsum)*rstd
                            # kraw first: it gates the PE (LeakyAvg)
                            for i, tb in enumerate(range(g0, g1)):
                                tmp = evp.tile([128, CL], f32, tag="evt")
                                nc.vector.scalar_tensor_tensor(
                                    tmp[:], wsum_bc[:], negmu[tb][:],
                                    psk[i][:], ALU.mult, ALU.add)
                                nc.vector.tensor_scalar_mul(
                                    kraw[:, tb * CL:(tb + 1) * CL],
                                    tmp[:], rstd[tb][:])
                            for i, tb in enumerate(range(g0, g1)):
                                if is_ctx:
                                    tmp2 = evp.tile([128, CL], f32,
                                                    tag="evt2")
                                    nc.vector.scalar_tensor_tensor(
                                        tmp2[:], wvsum_bc[:], negmu[tb][:],
                                        psv[i][:], ALU.mult, ALU.add)
                                    nc.vector.tensor_scalar_mul(
                                        vraw[:, tb * CL:(tb + 1) * CL],
                                        tmp2[:], rstd[tb][:])

                # ---- v path (context only) ----
                vnorm = None
                if is_ctx:
                    vnorm = wp.tile([128, NTB * 528], f32r, tag="vnorm")
                    with tc.tile_pool(name="vtmp", bufs=3) as vtp, \
                         tc.tile_pool(name="vps", bufs=2,
                                      space="PSUM") as vpsp, \
                         tc.tile_pool(name="vst", bufs=3) as vsp:
                        for tb in range(NTB):
                            vr = vraw[:, tb * CL:(tb + 1) * CL].bitcast(f32)
                            # v_shift on the PE: S@v (+ E@v_next for row 127)
                            pvs = vpsp.tile([128, CL], f32, tag="pvs")
                            nc.tensor.matmul(
                                pvs[:], smat[:, 0:128],
                                vraw[:, tb * CL:(tb + 1) * CL],
                                start=True, stop=(tb == NTB - 1))
                            if tb < NTB - 1:
                                nc.tensor.matmul(
                                    pvs[:], smat[:, 128:256],
                                    vraw[:, (tb + 1) * CL:(tb + 2) * CL],
                                    start=False, stop=True)
                            vsh = vtp.tile([128, CL], f32, tag="vsh")
                            # vmix = vraw + (1-coef)*(vsh - vraw), in place
                            nc.vector.tensor_tensor(vsh[:], pvs[:], vr,
                                                    ALU.subtract)
                            c_b = coef1[:].unsqueeze(2).broadcast_to(
                                (128, NHL, HS))
                            nc.vector.tensor_tensor(
                                vsh[:].rearrange("p (h d) -> p h d", h=NHL),
                                vsh[:].rearrange("p (h d) -> p h d", h=NHL),
                                c_b, ALU.mult)
                            nc.vector.tensor_tensor(vsh[:], vsh[:], vr,
                                                    ALU.add)
                            sq = vtp.tile([128, CL], f32, tag="vsq")
                            nc.vector.tensor_tensor(sq[:], vsh[:], vsh[:],
                                                    ALU.mult)
                            ssq = vsp.tile([128, NHL], f32, tag="vssq")
                            nc.vector.reduce_sum(
                                ssq[:], sq[:].rearrange("p (h d) -> p h d",
                                                        h=NHL), axis=AXL.X)
                            lnv = vsp.tile([128, NHL], f32, tag="vlnv")
                            nc.scalar.activation(lnv[:], ssq[:], AF.Ln)
                            rn = vsp.tile([128, NHL], f32, tag="vrn")
                            nc.scalar.activation(rn[:], lnv[:], AF.Exp,
                                                 scale=-0.5)
                            rns = vsp.tile([128, NHL], f32, tag="vrns")
                            nc.vector.tensor_tensor(rns[:], rn[:], vsv[:],
                                                    ALU.mult)
                            rns_b = rns[:].unsqueeze(2).broadcast_to(
                                (128, NHL, HS))
                            vslice = vnorm[:, tb * 528:(tb + 1) * 528]
                            nc.vector.tensor_tensor(
                                vslice.rearrange("p (h c) -> p h c",
                                                 h=NHL)[:, :, 0:64],
                                vsh[:].rearrange("p (h d) -> p h d", h=NHL),
                                rns_b, ALU.mult)
                            nc.sync.dma_start(
                                vslice.rearrange("p (h c) -> p h c",
                                                 h=NHL)[:, :, 64:66],
                                vpad[:].rearrange("p (h t) -> p h t", h=NHL))

                # ---- LeakyAvg + normalize + transpose -> ktall ----
                dmat = wp.tile([128, NHL * 256], f32r, tag="dmat")
                nc.sync.dma_start(dmat[:], dmat_in[:].bitcast(f32r))
                ktall = wp.tile([128, 4096], f32r, tag="ktall")
                with tc.tile_pool(name=f"lv{branch_id}", bufs=2,
                                  space="PSUM") as lvp, \
                     tc.tile_pool(name=f"ltr{branch_id}", bufs=2,
                                  space="PSUM") as ltp, \
                     tc.tile_pool(name=f"le{branch_id}", bufs=3) as lep, \
                     tc.tile_pool(name=f"ls{branch_id}", bufs=4) as lsp:
                    for h in range(NHL):
                        pl = lvp.tile([128, CL], f32, tag="pl")
                        kview = kraw[:].rearrange("p (b r) -> p b r", r=CL)
                        rhs_all = kview[:, :, h * 64:h * 64 + 64]
                        nc.tensor.matmul(
                            pl[:], dmat[:, h * 256:h * 256 + 128],
                            rhs_all, start=True, stop=False)
                        rhs_prev = kview[:, 0:7, h * 64:h * 64 + 64]
                        nc.tensor.matmul(
                            pl[:, 64:512], dmat[:, h * 256 + 128:h * 256 + 256],
                            rhs_prev, start=False, stop=True)
                        lraw = lep.tile([128, CL], f32, tag="lraw")
                        nc.scalar.copy(lraw[:], pl[:])
                        sq = lep.tile([128, CL], f32, tag="lsq")
                        nc.vector.tensor_tensor(sq[:], lraw[:], lraw[:],
                                                ALU.mult)
                        ssq = lsp.tile([128, 8], f32, tag="lssq")
                        nc.vector.reduce_sum(
                            ssq[:], sq[:].rearrange("p (b d) -> p b d", d=64),
                            axis=AXL.X)
                        lnv = lsp.tile([128, 8], f32, tag="llnv")
                        nc.scalar.activation(lnv[:], ssq[:], AF.Ln)
                        rn = lsp.tile([128, 8], f32, tag="lrn")
                        nc.scalar.activation(rn[:], lnv[:], AF.Exp, scale=-0.5)
                        rns = lsp.tile([128, 8], f32, tag="lrns")
                        nc.vector.tensor_scalar_mul(rns[:], rn[:],
                                                    svec[:, h:h + 1])
                        kfeat = lep.tile([128, CL], f32r, tag="kfeat")
                        rb = rns[:].unsqueeze(2).broadcast_to((128, 8, 64))
                        nc.vector.tensor_tensor(
                            kfeat[:].rearrange("p (b d) -> p b d", d=64),
                            lraw[:].rearrange("p (b d) -> p b d", d=64),
                            rb, ALU.mult)
                        # transpose 8 blocks of (128,64) -> (64,128)
                        pbase = (h % 2) * 64
                        fbase = (h // 2) * 1024
                        for half in range(2):
                            ptr = ltp.tile([64, 512], f32, tag="ktr")
                            for q in range(4):
                                blk = half * 4 + q
                                nc.tensor.transpose(
                                    ptr[:, q * 128:(q + 1) * 128]
                                    .bitcast(f32r),
                                    kfeat[:, blk * 64:(blk + 1) * 64],
                                    ident[:])
                            nc.scalar.copy(
                                ktall[pbase:pbase + 64,
                                      fbase + half * 512:fbase + half * 512
                                      + 512],
                                ptr[:])

                # ---- attention ----
                ytall = wp.tile([128, 4096], f32r, tag="ytall")
                with tc.tile_pool(name=f"as{branch_id}", bufs=4,
                                  space="PSUM") as asp, \
                     tc.tile_pool(name=f"ay{branch_id}", bufs=3,
                                  space="PSUM") as ayp, \
                     tc.tile_pool(name=f"ab{branch_id}", bufs=1,
                                  space="PSUM") as abp, \
                     tc.tile_pool(name=f"at{branch_id}", bufs=5) as atp, \
                     tc.tile_pool(name=f"ar{branch_id}", bufs=2) as arp:
                    for h in range(NHL):
                        pbase = (h % 2) * 64
                        fbase = (h // 2) * 1024
                        kt_h = ktall[pbase:pbase + 64, fbase:fbase + 1024]
                        for qc in range(2):
                            py = ayp.tile([66, 512], f32, tag="py")
                            njb = 4 if (is_ctx and qc == 0) else 8
                            for jb in range(njb):
                                ps = asp.tile([128, 512], f32, tag="ps")
                                if is_ctx:
                                    lhs_sc = kt_h[:, jb * 128:(jb + 1) * 128]
                                else:
                                    lhs_sc = pktall[pbase:pbase + 64,
                                                    fbase + jb * 128:
                                                    fbase + (jb + 1) * 128]
                                nc.tensor.matmul(
                                    ps[:], lhs_sc,
                                    kt_h[:, qc * 512:(qc + 1) * 512],
                                    start=True, stop=True)
                                att = atp.tile([128, 512], f32r, tag="att")
                                r = jb - qc * 4
                                if is_ctx and r >= 0:
                                    araw = atp.tile([128, 512], f32,
                                                    tag="araw")
                                    nc.scalar.activation(araw[:], ps[:],
                                                         AF.Exp)
                                    nc.vector.tensor_tensor(
                                        att[:], araw[:],
                                        maskt[:, r * 512:(r + 1) * 512],
                                        ALU.mult)
                                else:
                                    nc.scalar.activation(att[:], ps[:],
                                                         AF.Exp)
                                if is_ctx:
                                    lhs_v = vnorm[:, jb * 528 + h * 66:
                                                  jb * 528 + (h + 1) * 66]
                                else:
                                    lhs_v = pvo[:, h * 528 + jb * 66:
                                                h * 528 + (jb + 1) * 66]
                                nc.tensor.matmul(py[:], lhs_v, att[:],
                                                 start=(jb == 0),
                                                 stop=(jb == njb - 1))
                            # 1/denom = exp(-ln(denom)) on ScalarE (2 ULP;
                            # stays in the natural_log_exp table set)
                            lrow = arp.tile([1, 512], f32, tag="lrow")
                            nc.scalar.activation(lrow[:], py[64:65, :], AF.Ln)
                            rrow = arp.tile([1, 512], f32r, tag="rrow")
                            nc.scalar.activation(rrow[:], lrow[:], AF.Exp,
                                                 scale=-1.0)
                            pb = abp.tile([64, 512], f32, tag="pb")
                            nc.tensor.matmul(pb[:], ones1[:], rrow[:],
                                             start=True, stop=True)
                            bcs = atp.tile([64, 512], f32, tag="bcs")
                            nc.scalar.copy(bcs[:], pb[:])
                            nc.vector.tensor_tensor(
                                ytall[pbase:pbase + 64,
                                      fbase + qc * 512:fbase + qc * 512 + 512],
                                py[0:64, :], bcs[:], ALU.mult)
                        if is_ctx:
                            # zero out the t=0 column (query 0 has no keys)
                            nc.sync.dma_start(
                                ytall[pbase:pbase + 64, fbase:fbase + 1],
                                zcol[0:64, :])

                # ---- c_proj -> collective -> residual into X ----
                # context: chunked AllReduce (both cores need full x').
                # persistent: ReduceScatter; each core finishes only its
                # half of the output rows (host stitches halves).
                with tc.tile_pool(name=f"cp{branch_id}", bufs=2,
                                  space="PSUM") as cpp, \
                     tc.tile_pool(name=f"cw{branch_id}", bufs=2) as cwp, \
                     tc.tile_pool(name=f"cs{branch_id}", bufs=2) as csp:
                    cpw = [None] * 4
                    for cb in range(4):
                        cpw[cb] = cwp.tile([128, 1024], f32r, name=f"cpw{cb}", tag=f"cpw{cb}")
                        nc.sync.dma_start(
                            cpw[cb][:],
                            cp_in[cb * 128:(cb + 1) * 128, :].bitcast(f32r))
                    for tb in range(NTB):
                        stage = csp.tile([128, 1024], f32, tag="cstage")
                        for co in range(2):
                            pc = cpp.tile([128, 512], f32, tag="pc")
                            for cb in range(4):
                                nc.tensor.matmul(
                                    pc[:],
                                    ytall[:, cb * 1024 + tb * 128:
                                          cb * 1024 + tb * 128 + 128],
                                    cpw[cb][:, co * 512:(co + 1) * 512],
                                    start=(cb == 0), stop=(cb == 3))
                            nc.scalar.copy(stage[:, co * 512:(co + 1) * 512],
                                           pc[:])
                        nc.sync.dma_start(
                            cc_in_t[tb * 128:(tb + 1) * 128, :], stage[:])
                        if is_ctx and tb % 2 == 1:
                            # quarter-chunk AllReduce as rows are staged
                            lo = (tb - 1) * 128
                            nc.gpsimd.collective_compute(
                                "AllReduce", ALU.add, replica_groups=GROUPS,
                                ins=[cc_in_t[lo:lo + 256, :]],
                                outs=[cc_out_t[lo:lo + 256, :]])
                    if is_ctx:
                        for tb in range(NTB):
                            back = csp.tile([128, 1024], f32, tag="cback")
                            nc.sync.dma_start(
                                back[:], cc_out_t[tb * 128:(tb + 1) * 128, :])
                            nc.vector.tensor_tensor(
                                X[:, tb * 1024:(tb + 1) * 1024],
                                X[:, tb * 1024:(tb + 1) * 1024].bitcast(f32),
                                back[:], ALU.add)
                    else:
                        # each core gets its pair-rank's half of the summed
                        # pm; host adds it onto the matching half of x'.
                        nc.gpsimd.collective_compute(
                            "ReduceScatter", ALU.add, replica_groups=GROUPS,
                            ins=[cc_in_t[:]], outs=[cc_out_t[0:512, :]])
                        for i in range(4):
                            back = csp.tile([128, 1024], f32, tag="cback")
                            nc.sync.dma_start(
                                back[:], cc_out_t[i * 128:(i + 1) * 128, :])
                            nc.sync.dma_start(
                                pm_out[i * 128:(i + 1) * 128, :], back[:])

            # -------- context branch --------
            pktall = None
            branch(0, wka_in, wsb["wka"], dmata_in, sveca, cc1_in, cc1_out,
                   wv_in=wva_in, wvsum_bc=wsb["wva"], cp_in=cpa_in)

            # x' is final except for the pm half-add done on host
            nc.sync.dma_start(
                xp_out[:].bitcast(f32r).rearrange("(b p) c -> p b c", p=128),
                X[:].rearrange("p (b c) -> p b c", b=NTB))

            # load persistent-memory keys into the vnorm slot (context-only)
            pktall = wp.tile([128, 4224], f32r, tag="vnorm")
            nc.sync.dma_start(pktall[:, 0:4096], pkt_in[:].bitcast(f32r))

            # -------- persistent branch --------
            branch(1, wkm_in, wsb["wkm"], dmatm_in, svecm, cc2_in, cc2_out,
                   cp_in=cpm_in)



    return nc


_prog_cache = {}


def _get_program():
    if "nc" not in _prog_cache:
        _prog_cache["nc"] = _build_program()
    return _prog_cache["nc"]


def _host_prep(inputs):
    """Build the 8 per-core input maps from the full-problem inputs."""
    x = np.asarray(inputs["x"], np.float32)
    ln1 = np.asarray(inputs["ln1_w"], np.float32)
    ln2 = np.asarray(inputs["ln2_w"], np.float32)
    Wk_a = np.asarray(inputs["Wk_a"], np.float32)
    Wv_a = np.asarray(inputs["Wv_a"], np.float32)
    cproj_a = np.asarray(inputs["cproj_a"], np.float32)
    beta_a = np.asarray(inputs["beta_a"], np.float32).reshape(NH)
    kscale_a = np.asarray(inputs["kscale_a"], np.float32).reshape(NH)
    vcoef = np.asarray(inputs["vcoef"], np.float32).reshape(NH)
    vscale = np.asarray(inputs["vscale"], np.float32).reshape(NH)
    Wk_m = np.asarray(inputs["Wk_m"], np.float32)
    beta_m = np.asarray(inputs["beta_m"], np.float32).reshape(NH)
    kscale_m = np.asarray(inputs["kscale_m"], np.float32).reshape(NH)
    Pk = np.asarray(inputs["Pk"], np.float32)
    Pv = np.asarray(inputs["Pv"], np.float32)
    out_scale = np.asarray(inputs["out_scale"], np.float32).reshape(NH)
    cproj_m = np.asarray(inputs["cproj_m"], np.float32)

    J, I = np.meshgrid(np.arange(128), np.arange(128), indexing="ij")

    def dmats(beta, heads):
        out = np.zeros((128, NHL * 256), np.float32)
        for i, h in enumerate(heads):
            b = abs(float(beta[h])) * EXP_SCALING
            out[:, i * 256:i * 256 + 128] = np.where(
                I >= J, np.exp(-(I - J) * b), 0.0)
            out[:, i * 256 + 128:i * 256 + 256] = np.exp(-((I + 128) - J) * b)
        return out

    # context diagonal masks: mask_r[jl, ql] = 1 if jl + r*128 < ql
    maskt = np.zeros((128, 2048), np.float32)
    jl = np.arange(128)[:, None]
    ql = np.arange(512)[None, :]
    for r in range(4):
        maskt[:, r * 512:(r + 1) * 512] = (jl + r * 128 < ql)

    vpad = np.zeros((128, 16), np.float32)
    vpad[:, 0::2] = 1.0

    def _smat():
        st = np.eye(128, k=-1, dtype=np.float32)   # S_T[j,t]=1 iff j==t+1
        e = np.zeros((128, 128), np.float32)
        e[0, 127] = 1.0                            # row127 <- next block row0
        return np.concatenate([st, e], axis=1)

    base = {
        "ident": np.eye(128, dtype=np.float32),
        "maskt": maskt,
        "ones1": np.ones((1, 64), np.float32),
        "vpad": vpad,
        "zcol": np.zeros((128, 1), np.float32),
        "epsv": np.full((128, 1), 1e-5, np.float32),
        "zrow": np.zeros((1, 512), np.float32),
        "smat": _smat(),
    }

    in_maps = []
    for c in range(N_CORES):
        b = c // 2
        hh = c % 2
        cols = slice(hh * CL, (hh + 1) * CL)
        heads = list(range(hh * NHL, hh * NHL + NHL))

        wka = (Wk_a * ln1[None, :])[cols].T.copy()      # (C, 512)
        wva = (Wv_a * ln1[None, :])[cols].T.copy()
        wkm = (Wk_m * ln2[None, :])[cols].T.copy()

        sva = np.exp(np.minimum(1.0 * EXP_SCALING * kscale_a[heads],
                                KSCALE_MAX))
        svm = np.exp(np.minimum(2.0 * EXP_SCALING * kscale_m[heads],
                                KSCALE_MAX))
        vs = np.exp(EXP_SCALING * vscale[heads])
        c1 = 1.0 - vcoef[heads]
        osc = np.exp(EXP_SCALING * out_scale[heads]) / Pk.shape[0]

        pkt = np.zeros((128, 4096), np.float32)
        pvo = np.zeros((128, NHL * 8 * 66), np.float32)
        for i, h in enumerate(heads):
            pb_ = (i % 2) * 64
            fb = (i // 2) * 1024
            pkt[pb_:pb_ + 64, fb:fb + 1024] = Pk[0, 0, h].T
            for pb2 in range(8):
                col = i * 528 + pb2 * 66
                pvo[:, col:col + 64] = Pv[0, 0, h, pb2 * 128:(pb2 + 1) * 128,
                                          :] * osc[i]
                pvo[:, col + 64] = 1.0
                pvo[:, col + 65] = 0.0

        m = dict(base)
        m.update({
            "x": np.ascontiguousarray(x[b]),
            "wka": np.ascontiguousarray(wka),
            "wva": np.ascontiguousarray(wva),
            "wkm": np.ascontiguousarray(wkm),
            "cpa": np.ascontiguousarray(cproj_a[:, cols].T),
            "cpm": np.ascontiguousarray(cproj_m[:, cols].T),
            "wkasb": np.broadcast_to(wka.sum(0), (128, CL)).copy(),
            "wvasb": np.broadcast_to(wva.sum(0), (128, CL)).copy(),
            "wkmsb": np.broadcast_to(wkm.sum(0), (128, CL)).copy(),
            "dmata": dmats(beta_a, heads),
            "dmatm": dmats(beta_m, heads),
            "sveca": np.broadcast_to(sva, (128, NHL)).copy(),
            "svecm": np.broadcast_to(svm, (128, NHL)).copy(),
            "coef1": np.broadcast_to(c1, (128, NHL)).copy(),
            "vs": np.broadcast_to(vs, (128, NHL)).copy(),
            "pkt": pkt,
            "pvo": pvo,
        })
        in_maps.append(m)
    return in_maps


def _assemble(res):
    out = np.empty((B, T, C), np.float32)
    for b in range(B):
        xp = res.results[2 * b]["xp"]
        out[b] = xp
        out[b, 0:512] += res.results[2 * b]["pm"]
        out[b, 512:1024] += res.results[2 * b + 1]["pm"]
    return out


def kernel(**inputs):
    nc = _get_program()
    in_maps = _host_prep(inputs)
    res = run_bass_kernel_spmd(nc, in_maps, list(range(N_CORES)))
    return _assemble(res)


def kernel_traced(**inputs):
    """Like kernel() but returns (out, BassKernelResults) with HW timing."""
    nc = _get_program()
    in_maps = _host_prep(inputs)
    res = run_bass_kernel_spmd(nc, in_maps, list(range(N_CORES)), trace=True)
    return _assemble(res), res


# revision 23
# speedup vs baseline: 1.2056x; 1.0016x over previous
"""Trainium2 Bass kernel for nn_Block_75840532513234 (dense transformer block).

Sharding: 8 cores; core c handles batch b = c//2 and head-half hh = c%2
(8 of 16 heads). The two cores of a pair all-reduce the c_proj partial sums
(row-sharded c_proj) after each of the two sub-blocks, so both hold the full
residual stream; the pair's even core's output is used.

Layout strategy (per core):
  - x, x' kept natural (t-part, c-free); LN stats per-partition.
  - LN mean/scale folded into projection evictions as a rank-1 correction
    ((x@W - mu*colsum(W)) * rstd), so x is transposed raw on the PE.
  - LeakyAvg = banded matmul with two 128x128 per-head decay matrices
    (exp decay below ~1e-28 beyond 128 steps for beta >= 0.5).
  - Attention computed fully transposed: scoresT = KT.T-block @ KT,
    attT = exp(scoresT) (softmax needs no max subtraction: |scores| <= ~10),
    YT = [V|1].T @ attT gives Y and the denominators in one accumulation.
  - ln1/ln2 weights folded into W_k/W_v host-side; kscale via svec;
    out_scale/PM_COUNT folded into Pv host-side; vscale into v-normalize.
"""
import sys
import math

sys.path.insert(0, '/opt/trn_rl_repo')

import numpy as np

# ---------------------------------------------------------------------------
# Patches for this container's walrus build: it allows only ONE sync-wait per
# instruction, while Tile attaches several (final drain; ldweights). Split the
# extras onto standalone single-wait EventSemaphore instructions.
# ---------------------------------------------------------------------------
import concourse.tile as tile
import concourse.bass as bass
from concourse import mybir
from concourse.vector_clock import ScopedClock

_ev_ctr = [0]


def _split_multi_waits(nc):
    for f in nc.m.functions:
        for bb in f.blocks:
            il = bb.instructions
            i = 0
            while i < len(il):
                inst = il[i]
                si = inst.sync_info
                if si is not None and si.on_wait and len(si.on_wait) > 1:
                    waits = list(si.on_wait)
                    si.on_wait.clear()
                    si.on_wait.append(waits[-1])
                    for w in waits[:-1]:
                        _ev_ctr[0] += 1
                        ev = mybir.InstEventSemaphore(
                            name=f"EVSPLIT-{_ev_ctr[0]}", ins=[], outs=[])
                        ev.engine = inst.engine
                        ev.sync_info = mybir.SyncInfo(on_wait=[], on_update=[])
                        ev.sync_info.on_wait.append(w)
                        il.insert(i, ev)
                        i += 1
                i += 1


def _patched_drain_and_barrier(self, tick_clock, wait_clock):
    nc = self.nc
    drain_inst = nc.sync.drain()
    wait_clock.add_sem_waits(
        drain_inst.ins, ScopedClock({None: tick_clock.global_clock}))
    nc.all_engine_barrier()
    popped = nc._tile_sem_poison_stack.pop()
    assert popped is self._sem_poison
    nc.clear_and_free_semaphores(list(self.sems.allocated().values()))
    nc.all_engine_barrier()


tile.TileContext._drain_and_barrier = _patched_drain_and_barrier

_orig_tile_exit = tile.TileContext.__exit__


def _patched_tile_exit(self, *a, **k):
    r = _orig_tile_exit(self, *a, **k)
    _split_multi_waits(self.nc)
    return r


tile.TileContext.__exit__ = _patched_tile_exit

# NTFF profile hook (trimmed image lacks antenv.axon_hooks).
import types as _types

if "antenv.axon_hooks" not in sys.modules:
    _m = _types.ModuleType("antenv.axon_hooks")
    _hook_store = [None]

    def _set_hook(h):
        _hook_store[0] = h

    def _get_hook():
        if _hook_store[0] is None:
            try:
                if '/root/.axon_site' not in sys.path:
                    sys.path.insert(0, '/root/.axon_site')
                from trn_agent_boot.trn_boot import _ntff_profile_via_ctypes
                _hook_store[0] = _ntff_profile_via_ctypes(
                    '/opt/axon/libaxon_pjrt.so')
            except Exception:
                return None
        return _hook_store[0]

    _m.set_axon_ntff_profile_hook = _set_hook
    _m.get_axon_ntff_profile_hook = _get_hook
    sys.modules["antenv.axon_hooks"] = _m
    import antenv as _antenv
    _antenv.axon_hooks = _m

from concourse.bass_utils import run_bass_kernel_spmd  # noqa: E402

# ---------------------------------------------------------------------------
# Problem constants (hardcoded per the grading contract)
# ---------------------------------------------------------------------------
B, T, C, NH = 4, 1024, 1024, 16
HS = C // NH              # 64
NHL = 8                   # heads per core
CL = NHL * HS             # 512 local channels
EXP_SCALING = 10.0
KSCALE_MAX = float(np.log(2.0 ** 16 - 1))
N_CORES = 8
GROUPS = [[0, 1], [2, 3], [4, 5], [6, 7]]

f32 = mybir.dt.float32
f32r = mybir.dt.float32r
AF = mybir.ActivationFunctionType
ALU = mybir.AluOpType
AXL = mybir.AxisListType

NTB = T // 128            # 8 t-blocks
NCB = C // 128            # 8 c-blocks


def _build_program():
    nc = bass.Bass(num_devices=N_CORES)

    # ---- I/O ----
    x_in = nc.dram_tensor("x", [T, C], f32, kind="ExternalInput")
    wka_in = nc.dram_tensor("wka", [C, CL], f32, kind="ExternalInput")
    wva_in = nc.dram_tensor("wva", [C, CL], f32, kind="ExternalInput")
    wkm_in = nc.dram_tensor("wkm", [C, CL], f32, kind="ExternalInput")
    cpa_in = nc.dram_tensor("cpa", [CL, C], f32, kind="ExternalInput")
    cpm_in = nc.dram_tensor("cpm", [CL, C], f32, kind="ExternalInput")
    wkasb_in = nc.dram_tensor("wkasb", [128, CL], f32, kind="ExternalInput")
    wvasb_in = nc.dram_tensor("wvasb", [128, CL], f32, kind="ExternalInput")
    wkmsb_in = nc.dram_tensor("wkmsb", [128, CL], f32, kind="ExternalInput")
    dmata_in = nc.dram_tensor("dmata", [128, NHL * 256], f32, kind="ExternalInput")
    dmatm_in = nc.dram_tensor("dmatm", [128, NHL * 256], f32, kind="ExternalInput")
    sveca_in = nc.dram_tensor("sveca", [128, NHL], f32, kind="ExternalInput")
    svecm_in = nc.dram_tensor("svecm", [128, NHL], f32, kind="ExternalInput")
    coef1_in = nc.dram_tensor("coef1", [128, NHL], f32, kind="ExternalInput")
    vs_in = nc.dram_tensor("vs", [128, NHL], f32, kind="ExternalInput")
    pkt_in = nc.dram_tensor("pkt", [128, 4096], f32, kind="ExternalInput")
    pvo_in = nc.dram_tensor("pvo", [128, NHL * 8 * 66], f32, kind="ExternalInput")
    ident_in = nc.dram_tensor("ident", [128, 128], f32, kind="ExternalInput")
    maskt_in = nc.dram_tensor("maskt", [128, 4 * 512], f32, kind="ExternalInput")
    ones1_in = nc.dram_tensor("ones1", [1, 64], f32, kind="ExternalInput")
    vpad_in = nc.dram_tensor("vpad", [128, 16], f32, kind="ExternalInput")
    zcol_in = nc.dram_tensor("zcol", [128, 1], f32, kind="ExternalInput")
    epsv_in = nc.dram_tensor("epsv", [128, 1], f32, kind="ExternalInput")
    zrow_in = nc.dram_tensor("zrow", [1, 512], f32, kind="ExternalInput")
    smat_in = nc.dram_tensor("smat", [128, 256], f32, kind="ExternalInput")

    xp_out = nc.dram_tensor("xp", [T, C], f32, kind="ExternalOutput")
    pm_out = nc.dram_tensor("pm", [512, C], f32, kind="ExternalOutput")

    cc1_in = nc.dram_tensor("cc1_in", [T, C], f32)
    cc1_out = nc.dram_tensor("cc1_out", [T, C], f32)
    cc2_in = nc.dram_tensor("cc2_in", [T, C], f32)
    cc2_out = nc.dram_tensor("cc2_out", [T, C], f32)

    with tile.TileContext(nc) as tc:
        # ---------------- persistent pools ----------------
        with tc.tile_pool(name="persist", bufs=1) as pp, \
             tc.tile_pool(name="work", bufs=1) as wp:
            # constants
            ident = pp.tile([128, 128], f32r, tag="ident")
            nc.sync.dma_start(ident[:], ident_in[:].bitcast(f32r))
            sveca = pp.tile([128, NHL], f32, tag="sveca")
            nc.sync.dma_start(sveca[:], sveca_in[:])
            svecm = pp.tile([128, NHL], f32, tag="svecm")
            nc.sync.dma_start(svecm[:], svecm_in[:])
            coef1 = pp.tile([128, NHL], f32, tag="coef1")
            nc.sync.dma_start(coef1[:], coef1_in[:])
            vsv = pp.tile([128, NHL], f32, tag="vsv")
            nc.sync.dma_start(vsv[:], vs_in[:])
            ones1 = pp.tile([1, 64], f32r, tag="ones1")
            nc.sync.dma_start(ones1[:], ones1_in[:].bitcast(f32r))
            vpad = pp.tile([128, 16], f32r, tag="vpad")
            nc.sync.dma_start(vpad[:], vpad_in[:].bitcast(f32r))
            zcol = pp.tile([128, 1], f32r, tag="zcol")
            nc.sync.dma_start(zcol[:], zcol_in[:].bitcast(f32r))
            epsv = pp.tile([128, 1], f32, tag="epsv")
            nc.sync.dma_start(epsv[:], epsv_in[:])
            smat = pp.tile([128, 256], f32r, tag="smat")
            nc.sync.dma_start(smat[:], smat_in[:].bitcast(f32r))
            maskt = pp.tile([128, 2048], f32, tag="maskt")
            nc.sync.dma_start(maskt[:], maskt_in[:])
            pvo = pp.tile([128, NHL * 8 * 66], f32r, tag="pvo")
            nc.sync.dma_start(pvo[:], pvo_in[:].bitcast(f32r))
            wsb = {}
            for nm, src in (("wka", wkasb_in), ("wva", wvasb_in),
                            ("wkm", wkmsb_in)):
                wsb[nm] = pp.tile([128, CL], f32, name=f"wsb_{nm}", tag=f"wsb_{nm}")
                nc.sync.dma_start(wsb[nm][:], src[:])

            # big persistent buffers (tags reused across the two branches)
            # X is f32r so PE can transpose it directly (rounds x once, ~1e-4)
            X = wp.tile([128, NTB * 1024], f32r, tag="X")        # x then out
            for tb in range(NTB):
                nc.sync.dma_start(
                    X[:, tb * 1024:(tb + 1) * 1024],
                    x_in[tb * 128:(tb + 1) * 128, :].bitcast(f32r))

            def branch(branch_id, wk_in, wsum_bc, dmat_in, svec, cc_in_t,
                       cc_out_t, wv_in=None, wvsum_bc=None, cp_in=None):
                """Emit one sub-block (context: with v; persistent: without).

                Returns nothing; accumulates residual into X in place.
                branch uses tiles tagged with shared tags so the second call
                reuses the first call's SBUF.
                """
                is_ctx = wv_in is not None

                # ---- LN stats ----
                negmu = []
                rstd = []
                with tc.tile_pool(name=f"st{branch_id}", bufs=2 * NTB) as sp, \
                     tc.tile_pool(name=f"sttmp{branch_id}", bufs=2) as stp:
                    for tb in range(NTB):
                        xs = X[:, tb * 1024:(tb + 1) * 1024].bitcast(f32)
                        s = stp.tile([128, 1], f32, tag="s")
                        nc.vector.reduce_sum(s[:], xs, axis=AXL.X)
                        sq = stp.tile([128, 1024], f32, tag="sq")
                        nc.vector.tensor_tensor(sq[:], xs, xs, ALU.mult)
                        ssq = stp.tile([128, 1], f32, tag="ssq")
                        nc.vector.reduce_sum(ssq[:], sq[:], axis=AXL.X)
                        nm = sp.tile([128, 1], f32, tag="negmu")
                        nc.vector.tensor_scalar_mul(nm[:], s[:], -1.0 / C)
                        mp = stp.tile([128, 1], f32, tag="mp")
                        nc.vector.tensor_scalar_mul(mp[:], s[:], 1.0 / C)
                        sn = stp.tile([128, 1], f32, tag="sn")
                        nc.vector.tensor_scalar_mul(sn[:], ssq[:], 1.0 / C)
                        var = stp.tile([128, 1], f32, tag="var")
                        nc.vector.scalar_tensor_tensor(
                            var[:], mp[:], nm[:], sn[:], ALU.mult, ALU.add)
                        lv = stp.tile([128, 1], f32, tag="lv")
                        nc.scalar.activation(lv[:], var[:], AF.Ln, bias=epsv[:])
                        rs = sp.tile([128, 1], f32, tag="rstd")
                        nc.scalar.activation(rs[:], lv[:], AF.Exp, scale=-0.5)
                        negmu.append(nm)
                        rstd.append(rs)

                    # ---- transpose + projections (grouped for PSUM) ----
                    kraw = wp.tile([128, NTB * CL], f32r, tag="kraw")
                    vraw = None
                    if is_ctx:
                        vraw = wp.tile([128, NTB * CL], f32r, tag="vraw")
                    tgroups = [(0, 3), (3, 6), (6, 8)]
                    with tc.tile_pool(name=f"pj{branch_id}", bufs=1,
                                      space="PSUM") as pjp, \
                         tc.tile_pool(name=f"ptr{branch_id}", bufs=2,
                                      space="PSUM") as ptp, \
                         tc.tile_pool(name=f"w{branch_id}", bufs=4) as wpool, \
                         tc.tile_pool(name=f"xtc{branch_id}", bufs=2) as xtp, \
                         tc.tile_pool(name=f"ev{branch_id}", bufs=3) as evp:
                        for g0, g1 in tgroups:
                            gn = g1 - g0
                            psk = [pjp.tile([128, CL], f32,
                                            name=f"psk{g0}_{i}",
                                            tag=f"psk{i}")
                                   for i in range(gn)]
                            psv = [pjp.tile([128, CL], f32,
                                            name=f"psv{g0}_{i}",
                                            tag=f"psv{i}")
                                   for i in range(gn)] if is_ctx else None
                            for cb in range(NCB):
                                xtc = xtp.tile([128, gn * 128], f32r,
                                               tag="xtc")
                                for i, tb in enumerate(range(g0, g1)):
                                    ptr = ptp.tile([128, 128], f32, tag="ptr")
                                    nc.tensor.transpose(
                                        ptr[:].bitcast(f32r),
                                        X[:, tb * 1024 + cb * 128:
                                          tb * 1024 + cb * 128 + 128],
                                        ident[:])
                                    nc.scalar.copy(
                                        xtc[:, i * 128:(i + 1) * 128],
                                        ptr[:])
                                wk_c = wpool.tile([128, CL], f32r, tag="wk")
                                nc.sync.dma_start(
                                    wk_c[:],
                                    wk_in[cb * 128:(cb + 1) * 128, :]
                                    .bitcast(f32r))
                                wv_c = None
                                if is_ctx:
                                    wv_c = wpool.tile([128, CL], f32r,
                                                      tag="wv")
                                    nc.sync.dma_start(
                                        wv_c[:],
                                        wv_in[cb * 128:(cb + 1) * 128, :]
                                        .bitcast(f32r))
                                for i in range(gn):
                                    nc.tensor.matmul(
                                        psk[i][:],
                                        xtc[:, i * 128:(i + 1) * 128],
                                        wk_c[:], start=(cb == 0),
                                        stop=(cb == NCB - 1))
                                    if is_ctx:
                                        nc.tensor.matmul(
                                            psv[i][:],
                                            xtc[:, i * 128:(i + 1) * 128],
                                            wv_c[:], start=(cb == 0),
                                            stop=(cb == NCB - 1))
                            # evict with LN fold: (psum + negmu*wsum)*rstd
                            # kraw first: it gates the PE (LeakyAvg)
                            for i, tb in enumerate(range(g0, g1)):
                                tmp = evp.tile([128, CL], f32, tag="evt")
                                nc.vector.scalar_tensor_tensor(
                                    tmp[:], wsum_bc[:], negmu[tb][:],
                                    psk[i][:], ALU.mult, ALU.add)
                                nc.vector.tensor_scalar_mul(
                                    kraw[:, tb * CL:(tb + 1) * CL],
                                    tmp[:], rstd[tb][:])
                            for i, tb in enumerate(range(g0, g1)):
                                if is_ctx:
                                    tmp2 = evp.tile([128, CL], f32,
                                                    tag="evt2")
                                    nc.vector.scalar_tensor_tensor(
                                        tmp2[:], wvsum_bc[:], negmu[tb][:],
                                        psv[i][:], ALU.mult, ALU.add)
                                    nc.vector.tensor_scalar_mul(
                                        vraw[:, tb * CL:(tb + 1) * CL],
                                        tmp2[:], rstd[tb][:])

                # ---- v path (context only) ----
                vnorm = None
                if is_ctx:
                    vnorm = wp.tile([128, NTB * 528], f32r, tag="vnorm")
                    with tc.tile_pool(name="vtmp", bufs=3) as vtp, \
                         tc.tile_pool(name="vps", bufs=2,
                                      space="PSUM") as vpsp, \
                         tc.tile_pool(name="vst", bufs=3) as vsp:
                        for tb in range(NTB):
                            vr = vraw[:, tb * CL:(tb + 1) * CL].bitcast(f32)
                            # v_shift on the PE: S@v (+ E@v_next for row 127)
                            pvs = vpsp.tile([128, CL], f32, tag="pvs")
                            nc.tensor.matmul(
                                pvs[:], smat[:, 0:128],
                                vraw[:, tb * CL:(tb + 1) * CL],
                                start=True, stop=(tb == NTB - 1))
                            if tb < NTB - 1:
                                nc.tensor.matmul(
                                    pvs[:], smat[:, 128:256],
                                    vraw[:, (tb + 1) * CL:(tb + 2) * CL],
                                    start=False, stop=True)
                            vsh = vtp.tile([128, CL], f32, tag="vsh")
                            # vmix = vraw + (1-coef)*(vsh - vraw), in place
                            nc.vector.tensor_tensor(vsh[:], pvs[:], vr,
                                                    ALU.subtract)
                            c_b = coef1[:].unsqueeze(2).broadcast_to(
                                (128, NHL, HS))
                            nc.vector.tensor_tensor(
                                vsh[:].rearrange("p (h d) -> p h d", h=NHL),
                                vsh[:].rearrange("p (h d) -> p h d", h=NHL),
                                c_b, ALU.mult)
                            nc.vector.tensor_tensor(vsh[:], vsh[:], vr,
                                                    ALU.add)
                            sq = vtp.tile([128, CL], f32, tag="vsq")
                            nc.vector.tensor_tensor(sq[:], vsh[:], vsh[:],
                                                    ALU.mult)
                            ssq = vsp.tile([128, NHL], f32, tag="vssq")
                            nc.vector.reduce_sum(
                                ssq[:], sq[:].rearrange("p (h d) -> p h d",
                                                        h=NHL), axis=AXL.X)
                            lnv = vsp.tile([128, NHL], f32, tag="vlnv")
                            nc.scalar.activation(lnv[:], ssq[:], AF.Ln)
                            rn = vsp.tile([128, NHL], f32, tag="vrn")
                            nc.scalar.activation(rn[:], lnv[:], AF.Exp,
                                                 scale=-0.5)
                            rns = vsp.tile([128, NHL], f32, tag="vrns")
                            nc.vector.tensor_tensor(rns[:], rn[:], vsv[:],
                                                    ALU.mult)
                            rns_b = rns[:].unsqueeze(2).broadcast_to(
                                (128, NHL, HS))
                            vslice = vnorm[:, tb * 528:(tb + 1) * 528]
                            nc.vector.tensor_tensor(
                                vslice.rearrange("p (h c) -> p h c",
                                                 h=NHL)[:, :, 0:64],
                                vsh[:].rearrange("p (h d) -> p h d", h=NHL),
                                rns_b, ALU.mult)
                            nc.sync.dma_start(
                                vslice.rearrange("p (h c) -> p h c",
                                                 h=NHL)[:, :, 64:66],
                                vpad[:].rearrange("p (h t) -> p h t", h=NHL))

                # ---- LeakyAvg + normalize + transpose -> ktall ----
                dmat = wp.tile([128, NHL * 256], f32r, tag="dmat")
                nc.sync.dma_start(dmat[:], dmat_in[:].bitcast(f32r))
                ktall = wp.tile([128, 4096], f32r, tag="ktall")
                with tc.tile_pool(name=f"lv{branch_id}", bufs=2,
                                  space="PSUM") as lvp, \
                     tc.tile_pool(name=f"ltr{branch_id}", bufs=2,
                                  space="PSUM") as ltp, \
                     tc.tile_pool(name=f"le{branch_id}", bufs=3) as lep, \
                     tc.tile_pool(name=f"ls{branch_id}", bufs=4) as lsp:
                    for h in range(NHL):
                        pl = lvp.tile([128, CL], f32, tag="pl")
                        kview = kraw[:].rearrange("p (b r) -> p b r", r=CL)
                        rhs_all = kview[:, :, h * 64:h * 64 + 64]
                        nc.tensor.matmul(
                            pl[:], dmat[:, h * 256:h * 256 + 128],
                            rhs_all, start=True, stop=False)
                        rhs_prev = kview[:, 0:7, h * 64:h * 64 + 64]
                        nc.tensor.matmul(
                            pl[:, 64:512], dmat[:, h * 256 + 128:h * 256 + 256],
                            rhs_prev, start=False, stop=True)
                        lraw = lep.tile([128, CL], f32, tag="lraw")
                        nc.scalar.copy(lraw[:], pl[:])
                        sq = lep.tile([128, CL], f32, tag="lsq")
                        nc.vector.tensor_tensor(sq[:], lraw[:], lraw[:],
                                                ALU.mult)
                        ssq = lsp.tile([128, 8], f32, tag="lssq")
                        nc.vector.reduce_sum(
                            ssq[:], sq[:].rearrange("p (b d) -> p b d", d=64),
                            axis=AXL.X)
                        lnv = lsp.tile([128, 8], f32, tag="llnv")
                        nc.scalar.activation(lnv[:], ssq[:], AF.Ln)
                        rn = lsp.tile([128, 8], f32, tag="lrn")
                        nc.scalar.activation(rn[:], lnv[:], AF.Exp, scale=-0.5)
                        rns = lsp.tile([128, 8], f32, tag="lrns")
                        nc.vector.tensor_scalar_mul(rns[:], rn[:],
                                                    svec[:, h:h + 1])
                        kfeat = lep.tile([128, CL], f32r, tag="kfeat")
                        rb = rns[:].unsqueeze(2).broadcast_to((128, 8, 64))
                        nc.vector.tensor_tensor(
                            kfeat[:].rearrange("p (b d) -> p b d", d=64),
                            lraw[:].rearrange("p (b d) -> p b d", d=64),
                            rb, ALU.mult)
                        # transpose 8 blocks of (128,64) -> (64,128)
                        pbase = (h % 2) * 64
                        fbase = (h // 2) * 1024
                        for half in range(2):
                            ptr = ltp.tile([64, 512], f32, tag="ktr")
                            for q in range(4):
                                blk = half * 4 + q
                                nc.tensor.transpose(
                                    ptr[:, q * 128:(q + 1) * 128]
                                    .bitcast(f32r),
                                    kfeat[:, blk * 64:(blk + 1) * 64],
                                    ident[:])
                            nc.scalar.copy(
                                ktall[pbase:pbase + 64,
                                      fbase + half * 512:fbase + half * 512
                                      + 512],
                                ptr[:])

                # ---- attention ----
                ytall = wp.tile([128, 4096], f32r, tag="ytall")
                with tc.tile_pool(name=f"as{branch_id}", bufs=4,
                                  space="PSUM") as asp, \
                     tc.tile_pool(name=f"ay{branch_id}", bufs=3,
                                  space="PSUM") as ayp, \
                     tc.tile_pool(name=f"ab{branch_id}", bufs=1,
                                  space="PSUM") as abp, \
                     tc.tile_pool(name=f"at{branch_id}", bufs=5) as atp, \
                     tc.tile_pool(name=f"ar{branch_id}", bufs=2) as arp:
                    for h in range(NHL):
                        pbase = (h % 2) * 64
                        fbase = (h // 2) * 1024
                        kt_h = ktall[pbase:pbase + 64, fbase:fbase + 1024]
                        for qc in range(2):
                            py = ayp.tile([66, 512], f32, tag="py")
                            njb = 4 if (is_ctx and qc == 0) else 8
                            for jb in range(njb):
                                ps = asp.tile([128, 512], f32, tag="ps")
                                if is_ctx:
                                    lhs_sc = kt_h[:, jb * 128:(jb + 1) * 128]
                                else:
                                    lhs_sc = pktall[pbase:pbase + 64,
                                                    fbase + jb * 128:
                                                    fbase + (jb + 1) * 128]
                                nc.tensor.matmul(
                                    ps[:], lhs_sc,
                                    kt_h[:, qc * 512:(qc + 1) * 512],
                                    start=True, stop=True)
                                att = atp.tile([128, 512], f32r, tag="att")
                                r = jb - qc * 4
                                if is_ctx and r >= 0:
                                    araw = atp.tile([128, 512], f32,
                                                    tag="araw")
                                    nc.scalar.activation(araw[:], ps[:],
                                                         AF.Exp)
                                    nc.vector.tensor_tensor(
                                        att[:], araw[:],
                                        maskt[:, r * 512:(r + 1) * 512],
                                        ALU.mult)
                                else:
                                    nc.scalar.activation(att[:], ps[:],
                                                         AF.Exp)
                                if is_ctx:
                                    lhs_v = vnorm[:, jb * 528 + h * 66:
                                                  jb * 528 + (h + 1) * 66]
                                else:
                                    lhs_v = pvo[:, h * 528 + jb * 66:
                                                h * 528 + (jb + 1) * 66]
                                nc.tensor.matmul(py[:], lhs_v, att[:],
                                                 start=(jb == 0),
                                                 stop=(jb == njb - 1))
                            # 1/denom = exp(-ln(denom)) on ScalarE (2 ULP;
                            # stays in the natural_log_exp table set)
                            lrow = arp.tile([1, 512], f32, tag="lrow")
                            nc.scalar.activation(lrow[:], py[64:65, :], AF.Ln)
                            rrow = arp.tile([1, 512], f32r, tag="rrow")
                            nc.scalar.activation(rrow[:], lrow[:], AF.Exp,
                                                 scale=-1.0)
                            pb = abp.tile([64, 512], f32, tag="pb")
                            nc.tensor.matmul(pb[:], ones1[:], rrow[:],
                                             start=True, stop=True)
                            bcs = atp.tile([64, 512], f32, tag="bcs")
                            nc.scalar.copy(bcs[:], pb[:])
                            nc.vector.tensor_tensor(
                                ytall[pbase:pbase + 64,
                                      fbase + qc * 512:fbase + qc * 512 + 512],
                                py[0:64, :], bcs[:], ALU.mult)
                        if is_ctx:
                            # zero out the t=0 column (query 0 has no keys)
                            nc.sync.dma_start(
                                ytall[pbase:pbase + 64, fbase:fbase + 1],
                                zcol[0:64, :])

                # ---- c_proj -> collective -> residual into X ----
                # context: chunked AllReduce (both cores need full x').
                # persistent: ReduceScatter; each core finishes only its
                # half of the output rows (host stitches halves).
                with tc.tile_pool(name=f"cp{branch_id}", bufs=2,
                                  space="PSUM") as cpp, \
                     tc.tile_pool(name=f"cw{branch_id}", bufs=2) as cwp, \
                     tc.tile_pool(name=f"cs{branch_id}", bufs=2) as csp:
                    cpw = [None] * 4
                    for cb in range(4):
                        cpw[cb] = cwp.tile([128, 1024], f32r, name=f"cpw{cb}", tag=f"cpw{cb}")
                        nc.sync.dma_start(
                            cpw[cb][:],
                            cp_in[cb * 128:(cb + 1) * 128, :].bitcast(f32r))
                    for tb in range(NTB):
                        stage = csp.tile([128, 1024], f32, tag="cstage")
                        for co in range(2):
                            pc = cpp.tile([128, 512], f32, tag="pc")
                            for cb in range(4):
                                nc.tensor.matmul(
                                    pc[:],
                                    ytall[:, cb * 1024 + tb * 128:
                                          cb * 1024 + tb * 128 + 128],
                                    cpw[cb][:, co * 512:(co + 1) * 512],
                                    start=(cb == 0), stop=(cb == 3))
                            nc.scalar.copy(stage[:, co * 512:(co + 1) * 512],
                                           pc[:])
                        nc.sync.dma_start(
                            cc_in_t[tb * 128:(tb + 1) * 128, :], stage[:])
                        if is_ctx and tb % 2 == 1:
                            # quarter-chunk AllReduce as rows are staged
                            lo = (tb - 1) * 128
                            nc.gpsimd.collective_compute(
                                "AllReduce", ALU.add, replica_groups=GROUPS,
                                ins=[cc_in_t[lo:lo + 256, :]],
                                outs=[cc_out_t[lo:lo + 256, :]])
                        if (not is_ctx) and tb == 3:
                            nc.gpsimd.collective_compute(
                                "ReduceScatter", ALU.add,
                                replica_groups=GROUPS,
                                ins=[cc_in_t[0:512, :]],
                                outs=[cc_out_t[0:256, :]])
                            for i in range(2):
                                backe = csp.tile([128, 1024], f32,
                                                 name=f"backe{i}",
                                                 tag="cback")
                                nc.sync.dma_start(
                                    backe[:],
                                    cc_out_t[i * 128:(i + 1) * 128, :])
                                nc.sync.dma_start(
                                    pm_out[i * 128:(i + 1) * 128, :],
                                    backe[:])
                    if is_ctx:
                        for tb in range(NTB):
                            back = csp.tile([128, 1024], f32, tag="cback")
                            nc.sync.dma_start(
                                back[:], cc_out_t[tb * 128:(tb + 1) * 128, :])
                            nc.vector.tensor_tensor(
                                X[:, tb * 1024:(tb + 1) * 1024],
                                X[:, tb * 1024:(tb + 1) * 1024].bitcast(f32),
                                back[:], ALU.add)
                    else:
                        # each core gets its pair-rank's quarter of each
                        # half-chunk; host stitches (see _assemble).
                        for ch in range(1, 2):
                            nc.gpsimd.collective_compute(
                                "ReduceScatter", ALU.add,
                                replica_groups=GROUPS,
                                ins=[cc_in_t[ch * 512:(ch + 1) * 512, :]],
                                outs=[cc_out_t[ch * 256:(ch + 1) * 256, :]])
                            for i in range(2):
                                back = csp.tile([128, 1024], f32,
                                                name=f"cback{ch}_{i}",
                                                tag="cback")
                                nc.sync.dma_start(
                                    back[:],
                                    cc_out_t[ch * 256 + i * 128:
                                             ch * 256 + (i + 1) * 128, :])
                                nc.sync.dma_start(
                                    pm_out[ch * 256 + i * 128:
                                           ch * 256 + (i + 1) * 128, :],
                                    back[:])

            # -------- context branch --------
            pktall = None
            branch(0, wka_in, wsb["wka"], dmata_in, sveca, cc1_in, cc1_out,
                   wv_in=wva_in, wvsum_bc=wsb["wva"], cp_in=cpa_in)

            # x' is final except for the pm half-add done on host
            nc.sync.dma_start(
                xp_out[:].bitcast(f32r).rearrange("(b p) c -> p b c", p=128),
                X[:].rearrange("p (b c) -> p b c", b=NTB))

            # load persistent-memory keys into the vnorm slot (context-only)
            pktall = wp.tile([128, 4224], f32r, tag="vnorm")
            nc.sync.dma_start(pktall[:, 0:4096], pkt_in[:].bitcast(f32r))

            # -------- persistent branch --------
            branch(1, wkm_in, wsb["wkm"], dmatm_in, svecm, cc2_in, cc2_out,
                   cp_in=cpm_in)



    return nc


_prog_cache = {}


def _get_program():
    if "nc" not in _prog_cache:
        _prog_cache["nc"] = _build_program()
    return _prog_cache["nc"]


def _host_prep(inputs):
    """Build the 8 per-core input maps from the full-problem inputs."""
    x = np.asarray(inputs["x"], np.float32)
    ln1 = np.asarray(inputs["ln1_w"], np.float32)
    ln2 = np.asarray(inputs["ln2_w"], np.float32)
    Wk_a = np.asarray(inputs["Wk_a"], np.float32)
    Wv_a = np.asarray(inputs["Wv_a"], np.float32)
    cproj_a = np.asarray(inputs["cproj_a"], np.float32)
    beta_a = np.asarray(inputs["beta_a"], np.float32).reshape(NH)
    kscale_a = np.asarray(inputs["kscale_a"], np.float32).reshape(NH)
    vcoef = np.asarray(inputs["vcoef"], np.float32).reshape(NH)
    vscale = np.asarray(inputs["vscale"], np.float32).reshape(NH)
    Wk_m = np.asarray(inputs["Wk_m"], np.float32)
    beta_m = np.asarray(inputs["beta_m"], np.float32).reshape(NH)
    kscale_m = np.asarray(inputs["kscale_m"], np.float32).reshape(NH)
    Pk = np.asarray(inputs["Pk"], np.float32)
    Pv = np.asarray(inputs["Pv"], np.float32)
    out_scale = np.asarray(inputs["out_scale"], np.float32).reshape(NH)
    cproj_m = np.asarray(inputs["cproj_m"], np.float32)

    J, I = np.meshgrid(np.arange(128), np.arange(128), indexing="ij")

    def dmats(beta, heads):
        out = np.zeros((128, NHL * 256), np.float32)
        for i, h in enumerate(heads):
            b = abs(float(beta[h])) * EXP_SCALING
            out[:, i * 256:i * 256 + 128] = np.where(
                I >= J, np.exp(-(I - J) * b), 0.0)
            out[:, i * 256 + 128:i * 256 + 256] = np.exp(-((I + 128) - J) * b)
        return out

    # context diagonal masks: mask_r[jl, ql] = 1 if jl + r*128 < ql
    maskt = np.zeros((128, 2048), np.float32)
    jl = np.arange(128)[:, None]
    ql = np.arange(512)[None, :]
    for r in range(4):
        maskt[:, r * 512:(r + 1) * 512] = (jl + r * 128 < ql)

    vpad = np.zeros((128, 16), np.float32)
    vpad[:, 0::2] = 1.0

    def _smat():
        st = np.eye(128, k=-1, dtype=np.float32)   # S_T[j,t]=1 iff j==t+1
        e = np.zeros((128, 128), np.float32)
        e[0, 127] = 1.0                            # row127 <- next block row0
        return np.concatenate([st, e], axis=1)

    base = {
        "ident": np.eye(128, dtype=np.float32),
        "maskt": maskt,
        "ones1": np.ones((1, 64), np.float32),
        "vpad": vpad,
        "zcol": np.zeros((128, 1), np.float32),
        "epsv": np.full((128, 1), 1e-5, np.float32),
        "zrow": np.zeros((1, 512), np.float32),
        "smat": _smat(),
    }

    in_maps = []
    for c in range(N_CORES):
        b = c // 2
        hh = c % 2
        cols = slice(hh * CL, (hh + 1) * CL)
        heads = list(range(hh * NHL, hh * NHL + NHL))

        wka = (Wk_a * ln1[None, :])[cols].T.copy()      # (C, 512)
        wva = (Wv_a * ln1[None, :])[cols].T.copy()
        wkm = (Wk_m * ln2[None, :])[cols].T.copy()

        sva = np.exp(np.minimum(1.0 * EXP_SCALING * kscale_a[heads],
                                KSCALE_MAX))
        svm = np.exp(np.minimum(2.0 * EXP_SCALING * kscale_m[heads],
                                KSCALE_MAX))
        vs = np.exp(EXP_SCALING * vscale[heads])
        c1 = 1.0 - vcoef[heads]
        osc = np.exp(EXP_SCALING * out_scale[heads]) / Pk.shape[0]

        pkt = np.zeros((128, 4096), np.float32)
        pvo = np.zeros((128, NHL * 8 * 66), np.float32)
        for i, h in enumerate(heads):
            pb_ = (i % 2) * 64
            fb = (i // 2) * 1024
            pkt[pb_:pb_ + 64, fb:fb + 1024] = Pk[0, 0, h].T
            for pb2 in range(8):
                col = i * 528 + pb2 * 66
                pvo[:, col:col + 64] = Pv[0, 0, h, pb2 * 128:(pb2 + 1) * 128,
                                          :] * osc[i]
                pvo[:, col + 64] = 1.0
                pvo[:, col + 65] = 0.0

        m = dict(base)
        m.update({
            "x": np.ascontiguousarray(x[b]),
            "wka": np.ascontiguousarray(wka),
            "wva": np.ascontiguousarray(wva),
            "wkm": np.ascontiguousarray(wkm),
            "cpa": np.ascontiguousarray(cproj_a[:, cols].T),
            "cpm": np.ascontiguousarray(cproj_m[:, cols].T),
            "wkasb": np.broadcast_to(wka.sum(0), (128, CL)).copy(),
            "wvasb": np.broadcast_to(wva.sum(0), (128, CL)).copy(),
            "wkmsb": np.broadcast_to(wkm.sum(0), (128, CL)).copy(),
            "dmata": dmats(beta_a, heads),
            "dmatm": dmats(beta_m, heads),
            "sveca": np.broadcast_to(sva, (128, NHL)).copy(),
            "svecm": np.broadcast_to(svm, (128, NHL)).copy(),
            "coef1": np.broadcast_to(c1, (128, NHL)).copy(),
            "vs": np.broadcast_to(vs, (128, NHL)).copy(),
            "pkt": pkt,
            "pvo": pvo,
        })
        in_maps.append(m)
    return in_maps


def _assemble(res):
    out = np.empty((B, T, C), np.float32)
    for b in range(B):
        out[b] = res.results[2 * b]["xp"]
        pm0 = res.results[2 * b]["pm"]
        pm1 = res.results[2 * b + 1]["pm"]
        out[b, 0:256] += pm0[0:256]
        out[b, 256:512] += pm1[0:256]
        out[b, 512:768] += pm0[256:512]
        out[b, 768:1024] += pm1[256:512]
    return out


def kernel(**inputs):
    nc = _get_program()
    in_maps = _host_prep(inputs)
    res = run_bass_kernel_spmd(nc, in_maps, list(range(N_CORES)))
    return _assemble(res)


def kernel_traced(**inputs):
    """Like kernel() but returns (out, BassKernelResults) with HW timing."""
    nc = _get_program()
    in_maps = _host_prep(inputs)
    res = run_bass_kernel_spmd(nc, in_maps, list(range(N_CORES)), trace=True)
    return _assemble(res), res
